# revision 33
# baseline (speedup 1.0000x reference)
"""Trainium2 Bass kernel for the GNN decoder (message passing, cond-layernorm).

Sharding: 8 cores = (batch b in {0,1}) x (pnode quarter q in {0..3}).
Each core owns pnode rows [q*16384, (q+1)*16384) of its batch and every edge
whose receiver lands in that range.  Edges are receiver-sorted on the host and
packed into NG groups of G=104 consecutive segments with a fixed budget of
EPG=512 edge slots per group (padded; pad slots have one-hot row == 0 so they
contribute nothing).

Host-side prep pre-gathers sender/receiver features per edge slot (sfT/rfT,
bf16, streamed from DRAM) so the device loop needs no gpsimd gathers.  The
edge loop is software-pipelined across groups (stages S1..S5 emitted with
skews 0/1/1/2/3) so each engine's in-order queue always has ready work; the
pnode loop is pipelined the same way.  Segment sums accumulate via one-hot
matmuls into SBUF-resident aggregate tables consumed directly by the pnode
phase.  Cond-norm affines are folded into downstream weights on device once
per launch.  The scalar engine only ever runs {silu, identity, copy} (one act
table set); layernorm rsqrt runs on the vector engine via a quake-style
bitcast seed plus one Newton step.  MLP bias rank-1 matmuls are emitted only
if the corresponding host-side bias vectors are nonzero (they are
structurally zero in this model).
"""

import numpy as np

import concourse.bass as bass
import concourse.tile as tile
from concourse import bacc
from concourse import mybir
from concourse.masks import make_identity

F32 = mybir.dt.float32
BF16 = mybir.dt.bfloat16
I32 = mybir.dt.int32

B, NR, NPTOT, E, F, EIN, H, OUT = 2, 16384, 65536, 262144, 128, 4, 16, 4
EPS = 1e-6
NQ = 4                  # pnode quarters per batch
QP = NPTOT // NQ        # pnodes per core (16384)
G = 104                 # segments per group
EPG = 512               # edge slots per group
NG = (QP + G - 1) // G  # groups per core (158)
NEP = NG * EPG          # padded edge slots per core
PB = 512                # pnode block width
NPB = QP // PB          # pnode blocks per core (32)
M = 2                   # groups per macro DMA batch

AF = mybir.ActivationFunctionType
ALU = mybir.AluOpType


def _build_nc(zero_bias):
    nc = bacc.Bacc("TRN2", target_bir_lowering=False, debug=False)

    def inp(name, shape, dtype=F32):
        return nc.dram_tensor(name, shape, dtype, kind="ExternalInput")

    efT_d = inp("efT", [EIN, NEP], BF16)
    sfT_d = inp("sfT", [F, NEP], BF16)
    rfT_d = inp("rfT", [F, NEP], BF16)
    oh_d = inp("ohM", [128, NG * 4 * G], BF16)
    pnT_d = inp("pnT", [F, QP], BF16)
    tau_d = inp("tau", [1, 1])
    inv_d = inp("invQ", [1, QP])
    m01_d = inp("m01Q", [1, QP], BF16)

    We1 = inp("We1", [EIN, F], BF16)
    be1 = inp("be1", [F, 1])
    We2 = inp("We2", [F, F], BF16)
    be2_4 = inp("be2_4", [1, 4 * F], BF16)
    Wu1a = inp("Wu1a", [F, F])
    Wu1b = inp("Wu1b", [F, F], BF16)
    Wu1c = inp("Wu1c", [F, F], BF16)
    bu1 = inp("bu1", [F, 1])
    Wu2 = inp("Wu2", [F, F], BF16)
    bu2_4 = inp("bu2_4", [1, 4 * F], BF16)
    Wp1n = inp("Wp1n", [F, F], BF16)
    Wp1g = inp("Wp1g", [F, F])
    bp1 = inp("bp1", [F, 1])
    Wp2 = inp("Wp2", [F, F], BF16)
    bp2_4 = inp("bp2_4", [1, 4 * F], BF16)
    Wo1 = inp("Wo1", [F, F])
    bo1 = inp("bo1", [F, 1])
    Wo2 = inp("Wo2", [F, OUT], BF16)
    bo2 = inp("bo2", [1, OUT], BF16)
    # conditioning nets: e(dge embed), u(pdate), p(node).  r is dead code.
    cond_w = {}
    for k in ("e", "u", "p"):
        cond_w[k] = (
            inp(f"C{k}1", [1, H]),
            inp(f"c{k}1", [H, 1]),
            inp(f"C{k}2a", [H, F]),     # scale half of C2
            inp(f"C{k}2b", [H, F]),     # shift half of C2
            inp(f"c{k}2a1", [F, 1]),    # c2[:F] + 1.0
            inp(f"c{k}2b", [F, 1]),     # c2[F:]
        )

    outT = nc.dram_tensor("outT", [OUT, QP], F32, kind="ExternalOutput")

    from contextlib import ExitStack

    with tile.TileContext(nc) as tc, ExitStack() as ctx:
        singles = ctx.enter_context(tc.tile_pool(name="singles", bufs=1))
        macro = ctx.enter_context(tc.tile_pool(name="macro", bufs=4))
        macroP = ctx.enter_context(tc.tile_pool(name="macroP", bufs=2))
        work = ctx.enter_context(tc.tile_pool(name="work", bufs=2))
        work3 = ctx.enter_context(tc.tile_pool(name="work3", bufs=3))
        small = ctx.enter_context(tc.tile_pool(name="small", bufs=4))
        psB = ctx.enter_context(tc.tile_pool(name="psB", bufs=2, space="PSUM"))
        psA = ctx.enter_context(tc.tile_pool(name="psA", bufs=2, space="PSUM"))
        psT = ctx.enter_context(tc.tile_pool(name="psT", bufs=2, space="PSUM"))
        psS = ctx.enter_context(tc.tile_pool(name="psS", bufs=2, space="PSUM"))

        # ---------- constants & resident tables ----------
        def load(name, dram, shape, dtype=F32):
            t = singles.tile(shape, dtype, tag=name)
            nc.sync.dma_start(out=t[:], in_=dram[:])
            return t

        pnT = load("pnT", pnT_d, [F, QP], BF16)
        sWe1 = load("We1", We1, [EIN, F], BF16)
        sbe1 = load("be1", be1, [F, 1])
        sWe2 = load("We2", We2, [F, F], BF16)
        sWu1a = load("Wu1a", Wu1a, [F, F])
        sWu1b = load("Wu1b", Wu1b, [F, F], BF16)
        sWu1c = load("Wu1c", Wu1c, [F, F], BF16)
        sbu1 = load("bu1", bu1, [F, 1])
        sWu2 = load("Wu2", Wu2, [F, F], BF16)
        sWp1n = load("Wp1n", Wp1n, [F, F], BF16)
        sWp1g = load("Wp1g", Wp1g, [F, F])
        sbp1 = load("bp1", bp1, [F, 1])
        sWp2 = load("Wp2", Wp2, [F, F], BF16)
        sWo1 = load("Wo1", Wo1, [F, F])
        sbo1 = load("bo1", bo1, [F, 1])
        sWo2 = load("Wo2", Wo2, [F, OUT], BF16)
        stau = load("tau", tau_d, [1, 1])
        if not zero_bias:
            sbe2_4 = load("be2_4", be2_4, [1, 4 * F], BF16)
            sbu2_4 = load("bu2_4", bu2_4, [1, 4 * F], BF16)
            sbp2_4 = load("bp2_4", bp2_4, [1, 4 * F], BF16)
            sbo2 = load("bo2", bo2, [1, OUT], BF16)

        ident = singles.tile([128, 128], F32, tag="ident")
        make_identity(nc, ident[:])
        ident16 = singles.tile([128, 128], BF16, tag="ident16")
        nc.vector.tensor_copy(out=ident16[:], in_=ident[:])
        ones_r = singles.tile([1, PB], BF16, tag="ones_r")
        nc.vector.memset(ones_r[:], 1.0)
        ones_r32 = singles.tile([1, 128], F32, tag="ones_r32")
        nc.vector.memset(ones_r32[:], 1.0)

        # SBUF-resident aggregation tables (written per group, read per block)
        aggS1 = singles.tile([128, NG * G], BF16, tag="aggS1")
        aggS2 = singles.tile([128, NG * G], BF16, tag="aggS2")

        magic = singles.tile([128, 8], I32, tag="magic")
        nc.vector.memset(magic[:], 0x5F3759DF)

        # ---------- conditioning nets (tau -> scale/shift) + weight folds ----------
        cvec = {}
        for k in ("e", "u", "p"):
            C1, c1, C2a, C2b, c2a1, c2b = cond_w[k]
            sC1 = load(f"C{k}1", C1, [1, H])
            sc1 = load(f"c{k}1", c1, [H, 1])
            sC2a = load(f"C{k}2a", C2a, [H, F])
            sC2b = load(f"C{k}2b", C2b, [H, F])
            sc2a1 = load(f"c{k}2a1", c2a1, [F, 1])
            sc2b = load(f"c{k}2b", c2b, [F, 1])

            ph = psS.tile([H, 1], F32, tag="pS")
            nc.tensor.matmul(ph[:], lhsT=sC1[:], rhs=stau[:], start=True, stop=True)
            hs = small.tile([H, 1], F32, tag=f"hs{k}")
            nc.scalar.activation(hs[:], ph[:], AF.Silu, bias=sc1[:], scale=1.0)

            pscale = psS.tile([F, 1], F32, tag="pS")
            nc.tensor.matmul(pscale[:], lhsT=sC2a[:], rhs=hs[:], start=True, stop=True)
            s1p = singles.tile([F, 1], F32, tag=f"s1p{k}")
            nc.vector.tensor_scalar(
                out=s1p[:], in0=pscale[:], scalar1=sc2a1[:], scalar2=None, op0=ALU.add
            )
            pshift = psS.tile([F, 1], F32, tag="pS")
            nc.tensor.matmul(pshift[:], lhsT=sC2b[:], rhs=hs[:], start=True, stop=True)
            shift = singles.tile([F, 1], F32, tag=f"shift{k}")
            nc.vector.tensor_scalar(
                out=shift[:], in0=pshift[:], scalar1=sc2b[:], scalar2=None, op0=ALU.add
            )
            cvec[k] = (s1p, shift)

        s1pe, shifte = cvec["e"]
        s1pu, shiftu = cvec["u"]
        s1pp, shiftp = cvec["p"]

        # fold cond-norm affines into downstream weights
        fWu1a = singles.tile([F, F], BF16, tag="fWu1a")
        nc.vector.tensor_tensor(
            out=fWu1a[:], in0=sWu1a[:], in1=s1pe[:].to_broadcast([F, F]), op=ALU.mult
        )
        pbu1 = psS.tile([F, 1], F32, tag="pS")
        nc.tensor.matmul(pbu1[:], lhsT=sWu1a[:], rhs=shifte[:], start=True, stop=True)
        fbu1 = singles.tile([F, 1], F32, tag="fbu1")
        nc.vector.tensor_scalar(
            out=fbu1[:], in0=pbu1[:], scalar1=sbu1[:], scalar2=None, op0=ALU.add
        )

        fWp1ge = singles.tile([F, F], BF16, tag="fWp1ge")
        nc.vector.tensor_tensor(
            out=fWp1ge[:], in0=sWp1g[:], in1=s1pe[:].to_broadcast([F, F]), op=ALU.mult
        )
        fWp1gu = singles.tile([F, F], BF16, tag="fWp1gu")
        nc.vector.tensor_tensor(
            out=fWp1gu[:], in0=sWp1g[:], in1=s1pu[:].to_broadcast([F, F]), op=ALU.mult
        )
        shifteu = small.tile([F, 1], F32, tag="shifteu")
        nc.vector.tensor_tensor(
            out=shifteu[:], in0=shifte[:], in1=shiftu[:], op=ALU.add
        )
        pbpe = psS.tile([1, F], F32, tag="pS")
        nc.tensor.matmul(pbpe[:], lhsT=shifteu[:], rhs=sWp1g[:], start=True, stop=True)
        bpe_row = singles.tile([1, F], BF16, tag="bpe_row")
        nc.vector.tensor_copy(out=bpe_row[:], in_=pbpe[:])

        fWo1 = singles.tile([F, F], BF16, tag="fWo1")
        nc.vector.tensor_tensor(
            out=fWo1[:], in0=sWo1[:], in1=s1pp[:].to_broadcast([F, F]), op=ALU.mult
        )
        sWo116 = singles.tile([F, F], BF16, tag="sWo116")
        nc.vector.tensor_copy(out=sWo116[:], in_=sWo1[:])
        pbo1 = psS.tile([F, 1], F32, tag="pS")
        nc.tensor.matmul(pbo1[:], lhsT=sWo1[:], rhs=shiftp[:], start=True, stop=True)
        fbo1 = singles.tile([F, 1], F32, tag="fbo1")
        nc.vector.tensor_scalar(
            out=fbo1[:], in0=pbo1[:], scalar1=sbo1[:], scalar2=None, op0=ALU.add
        )

        # second MLP layer: 4 chunk matmuls (+ bias rank-1s only if nonzero)
        def emit_l2(psum4, ysrc, W, bias4):
            for c in range(4):
                nc.tensor.matmul(
                    psum4[:, c, :],
                    lhsT=ysrc[:, c * 128 : (c + 1) * 128],
                    rhs=W[:],
                    start=True,
                    stop=zero_bias,
                )
                if not zero_bias:
                    nc.tensor.matmul(
                        psum4[:, c, :], lhsT=ones_r[:, 0:128],
                        rhs=bias4[:, 0:128], start=False, stop=True,
                    )

        # layer-norm split: per-chunk bn stats into a shared mv tile, one
        # batched rsqrt chain (bitcast seed + 1 NR) for up to 2 LNs, then
        # per-chunk applies (3 on scalar via Identity, 1 on vector).
        magic8 = magic  # [128, 8] int32 0x5f3759df

        def ln_stats(psum4, mv8, half, tag):
            st = small.tile([128, 4, 6], F32, tag=f"st{tag}", name="st")
            for c in range(4):
                nc.vector.bn_stats(out=st[:, c, :], in_=psum4[:, c, :])
            for c in range(4):
                nc.vector.bn_aggr(out=mv8[:, half * 4 + c, :], in_=st[:, c, :])

        def ln_rsqrt(mv8, W, tag):
            vpe = small.tile([128, W], F32, tag=f"vp{tag}", name="vpe")
            nc.vector.tensor_scalar(
                out=vpe[:], in0=mv8[:, 0:W, 1], scalar1=EPS, scalar2=None,
                op0=ALU.add,
            )
            ish = small.tile([128, W], I32, tag=f"is{tag}", name="ish")
            nc.vector.tensor_scalar(
                out=ish[:], in0=vpe[:].bitcast(I32), scalar1=1, scalar2=None,
                op0=ALU.arith_shift_right,
            )
            y0i = small.tile([128, W], I32, tag=f"y0{tag}", name="y0i")
            nc.vector.tensor_tensor(
                out=y0i[:], in0=magic8[:, 0:W], in1=ish[:], op=ALU.subtract
            )
            y0 = y0i[:].bitcast(F32)
            y0sq = small.tile([128, W], F32, tag=f"yq{tag}", name="y0sq")
            nc.vector.tensor_tensor(out=y0sq[:], in0=y0, in1=y0, op=ALU.mult)
            th = small.tile([128, W], F32, tag=f"th{tag}", name="th")
            nc.vector.scalar_tensor_tensor(
                out=th[:], in0=y0sq[:], scalar=-0.5, in1=vpe[:],
                op0=ALU.mult, op1=ALU.mult,
            )
            rstd = small.tile([128, W], F32, tag=f"rs{tag}", name="rstd")
            nc.vector.scalar_tensor_tensor(
                out=rstd[:], in0=th[:], scalar=1.5, in1=y0,
                op0=ALU.add, op1=ALU.mult,
            )
            nmr = small.tile([128, W], F32, tag=f"nm{tag}", name="nmr")
            nc.vector.scalar_tensor_tensor(
                out=nmr[:], in0=mv8[:, 0:W, 0], scalar=-1.0, in1=rstd[:],
                op0=ALU.mult, op1=ALU.mult,
            )
            return rstd, nmr

        def ln_apply(psum4, out4, rstd, nmr, mv8, half):
            for c in range(4):
                w = half * 4 + c
                if c == 3:
                    nc.vector.tensor_scalar(
                        out=out4[:, c, :],
                        in0=psum4[:, c, :],
                        scalar1=mv8[:, w, 0:1],
                        scalar2=rstd[:, w : w + 1],
                        op0=ALU.subtract,
                        op1=ALU.mult,
                    )
                else:
                    nc.scalar.activation(
                        out4[:, c, :], psum4[:, c, :], AF.Identity,
                        bias=nmr[:, w : w + 1], scale=rstd[:, w : w + 1],
                    )

        # transpose [128, 4, 128] sbuf bf16 -> [128, 512] sbuf bf16 via the
        # XBAR DMA transpose (off the PE), issued from the scalar queue.
        def transp(ln4, outT_t):
            for c in range(4):
                nc.scalar.dma_start_transpose(
                    out=outT_t[:, c * 128 : (c + 1) * 128], in_=ln4[:, c, :]
                )

        # ---------- edge phase (software-pipelined across groups) ----------
        st_macro = {}   # macro index -> dict of macro tiles
        st_grp = {}     # group index -> dict of per-group tiles

        def macro_load(m):
            g0 = m * M
            gn = min(M, NG - g0)
            ew = gn * EPG
            t = {}
            t["ef"] = macro.tile([EIN, M * EPG], BF16, tag="efM", name="efM")
            nc.sync.dma_start(
                out=t["ef"][:, 0:ew], in_=efT_d[:, g0 * EPG : g0 * EPG + ew]
            )
            t["sf"] = macro.tile([F, M * EPG], BF16, tag="sfM", name="sfM")
            nc.sync.dma_start(
                out=t["sf"][:, 0:ew], in_=sfT_d[:, g0 * EPG : g0 * EPG + ew]
            )
            t["rf"] = macro.tile([F, M * EPG], BF16, tag="rfM", name="rfM")
            nc.sync.dma_start(
                out=t["rf"][:, 0:ew], in_=rfT_d[:, g0 * EPG : g0 * EPG + ew]
            )
            t["oh"] = macro.tile([128, M * 4 * G], BF16, tag="ohM", name="ohM")
            nc.sync.dma_start(
                out=t["oh"][:, 0 : gn * 4 * G],
                in_=oh_d[:, g0 * 4 * G : (g0 + gn) * 4 * G],
            )
            st_macro[m] = t

        def s1_embed(g):
            mt = st_macro[g // M]
            esl = slice((g % M) * EPG, (g % M + 1) * EPG)
            d = st_grp.setdefault(g, {})
            d["esl"] = esl
            pz1 = psB.tile([128, EPG], F32, tag="pB")
            nc.tensor.matmul(
                pz1[:], lhsT=sWe1[:], rhs=mt["ef"][:, esl], start=True, stop=True
            )
            y1 = work.tile([128, EPG], BF16, tag="y1")
            nc.scalar.activation(y1[:], pz1[:], AF.Silu, bias=sbe1[:], scale=1.0)
            pz2 = psA.tile([128, 4, 128], F32, tag="pA")
            emit_l2(pz2, y1, sWe2, None if zero_bias else sbe2_4)
            d["pz2"] = pz2

        def ln_driver(i):
            jobs = []
            if 1 <= i and i - 1 < NG:
                jobs.append(("e", i - 1))
            if 2 <= i and i - 2 < NG:
                jobs.append(("u", i - 2))
            if not jobs:
                return
            mv8 = small.tile([128, 8, 2], F32, tag="mv8", name="mv8")
            for h, (kind, g) in enumerate(jobs):
                d = st_grp[g]
                ln_stats(d["pz2" if kind == "e" else "pu2"], mv8, h, kind)
            rstd, nmr = ln_rsqrt(mv8, 4 * len(jobs), "eu")
            for h, (kind, g) in enumerate(jobs):
                d = st_grp[g]
                if kind == "e":
                    ln1 = work3.tile([128, 4, 128], BF16, tag="ln1", name="ln1")
                    ln_apply(d["pz2"], ln1, rstd, nmr, mv8, h)
                    d["ln1"] = ln1
                    del d["pz2"]
                    ln1T = work.tile([128, EPG], BF16, tag="ln1T", name="ln1T")
                    transp(ln1, ln1T)
                    d["ln1T"] = ln1T
                else:
                    ln2 = work.tile([128, 4, 128], BF16, tag="ln2", name="ln2")
                    ln_apply(d["pu2"], ln2, rstd, nmr, mv8, h)
                    d["ln2"] = ln2
                    del d["pu2"]

        def s3_update(g):
            d = st_grp[g]
            mt = st_macro[g // M]
            esl = d["esl"]
            pu1 = psB.tile([128, EPG], F32, tag="pB")
            nc.tensor.matmul(
                pu1[:], lhsT=fWu1a[:], rhs=d["ln1T"][:], start=True, stop=False
            )
            nc.tensor.matmul(
                pu1[:], lhsT=sWu1b[:], rhs=mt["sf"][:, esl], start=False, stop=False
            )
            nc.tensor.matmul(
                pu1[:], lhsT=sWu1c[:], rhs=mt["rf"][:, esl], start=False, stop=True
            )
            del d["ln1T"]
            yu = work.tile([128, EPG], BF16, tag="yu")
            nc.scalar.activation(yu[:], pu1[:], AF.Silu, bias=fbu1[:], scale=1.0)
            pu2 = psA.tile([128, 4, 128], F32, tag="pA")
            emit_l2(pu2, yu, sWu2, None if zero_bias else sbu2_4)
            d["pu2"] = pu2

        def s5_agg(g):
            d = st_grp[g]
            mt = st_macro[g // M]
            gm = g % M
            Sps = psS.tile([128, 2 * G], F32, tag="pS")
            for c in range(4):
                nc.tensor.matmul(
                    Sps[:, 0:G],
                    lhsT=d["ln1"][:, c, :],
                    rhs=mt["oh"][:, (gm * 4 + c) * G : (gm * 4 + c + 1) * G],
                    start=(c == 0),
                    stop=(c == 3),
                )
            for c in range(4):
                nc.tensor.matmul(
                    Sps[:, G : 2 * G],
                    lhsT=d["ln2"][:, c, :],
                    rhs=mt["oh"][:, (gm * 4 + c) * G : (gm * 4 + c + 1) * G],
                    start=(c == 0),
                    stop=(c == 3),
                )
            nc.vector.tensor_copy(
                out=aggS1[:, g * G : (g + 1) * G], in_=Sps[:, 0:G]
            )
            nc.scalar.copy(
                out=aggS2[:, g * G : (g + 1) * G], in_=Sps[:, G : 2 * G]
            )
            del st_grp[g]

        NMAC = (NG + M - 1) // M
        macro_load(0)
        for i in range(NG + 3):
            ln_driver(i)
            if i >= 3:
                s5_agg(i - 3)
            if i < NG:
                if i % M == 0 and (i // M) + 1 < NMAC:
                    macro_load(i // M + 1)
                s1_embed(i)
            if i >= 1 and i - 1 < NG:
                s3_update(i - 1)

        # ---------- pnode phase (software-pipelined across blocks) ----------
        OBW = 4  # blocks per staging window
        st_blk = {}

        def win_load(wi):
            w = {}
            w["inv"] = macroP.tile([1, OBW * PB], F32, tag="invW", name="invW")
            nc.sync.dma_start(
                out=w["inv"][:], in_=inv_d[:, wi * OBW * PB : (wi + 1) * OBW * PB]
            )
            w["m01"] = macroP.tile([1, OBW * PB], BF16, tag="m01W", name="m01W")
            nc.sync.dma_start(
                out=w["m01"][:], in_=m01_d[:, wi * OBW * PB : (wi + 1) * OBW * PB]
            )
            st_blk["w%d" % wi] = w

        def p1_front(j):
            d = st_blk.setdefault(j, {})
            sl = slice(j * PB, (j + 1) * PB)
            d["sl"] = sl
            if j % OBW == 0:
                if j == 0:
                    win_load(0)
                if j // OBW + 1 < NPB // OBW:
                    win_load(j // OBW + 1)
            w = st_blk["w%d" % (j // OBW)]
            wsl = slice((j % OBW) * PB, (j % OBW + 1) * PB)

            pinv = psT.tile([128, PB], F32, tag="pT")
            nc.tensor.matmul(
                pinv[:], lhsT=ones_r32[:], rhs=w["inv"][:, wsl],
                start=True, stop=True,
            )
            invb = work.tile([128, PB], F32, tag="invb")
            nc.scalar.copy(out=invb[:], in_=pinv[:])

            pA = psB.tile([128, PB], F32, tag="pB")
            nc.tensor.matmul(
                pA[:], lhsT=fWp1ge[:], rhs=aggS1[:, sl], start=True, stop=False
            )
            nc.tensor.matmul(
                pA[:], lhsT=fWp1gu[:], rhs=aggS2[:, sl], start=False, stop=True
            )
            tA = work.tile([128, PB], BF16, tag="tA")
            nc.vector.tensor_tensor(out=tA[:], in0=pA[:], in1=invb[:], op=ALU.mult)

            pzp = psB.tile([128, PB], F32, tag="pB")
            nc.tensor.matmul(
                pzp[:], lhsT=sWp1n[:], rhs=pnT[:, sl], start=True, stop=False
            )
            nc.tensor.matmul(
                pzp[:], lhsT=ident16[:], rhs=tA[:], start=False, stop=False
            )
            nc.tensor.matmul(
                pzp[:], lhsT=bpe_row[:], rhs=w["m01"][:, wsl],
                start=False, stop=True,
            )
            yp = work.tile([128, PB], BF16, tag="yp")
            nc.scalar.activation(yp[:], pzp[:], AF.Silu, bias=sbp1[:], scale=1.0)

            pp2 = psA.tile([128, 4, 128], F32, tag="pA")
            emit_l2(pp2, yp, sWp2, None if zero_bias else sbp2_4)
            d["pp2"] = pp2

        def p2_ln(j):
            d = st_blk[j]
            mv8 = small.tile([128, 8, 2], F32, tag="mv8", name="mv8")
            ln_stats(d["pp2"], mv8, 0, "p")
            rstd, nmr = ln_rsqrt(mv8, 4, "p")
            lnp = work3.tile([128, 4, 128], BF16, tag="ln1", name="lnp")
            ln_apply(d["pp2"], lnp, rstd, nmr, mv8, 0)
            del d["pp2"]
            lnpT = work.tile([128, PB], BF16, tag="ln1T", name="lnpT")
            transp(lnp, lnpT)
            d["lnpT"] = lnpT

        def p3_out(j):
            d = st_blk[j]
            sl = d["sl"]
            lnpT = d["lnpT"]
            pzo = psB.tile([128, PB], F32, tag="pB")
            nc.tensor.matmul(
                pzo[:], lhsT=fWo1[:], rhs=lnpT[:], start=True, stop=False
            )
            nc.tensor.matmul(
                pzo[:], lhsT=sWo116[:], rhs=pnT[:, sl], start=False, stop=True
            )
            yo = work.tile([128, PB], BF16, tag="yo")
            nc.scalar.activation(yo[:], pzo[:], AF.Silu, bias=fbo1[:], scale=1.0)

            po = psS.tile([OUT, PB], F32, tag="pS")
            nc.tensor.matmul(
                po[:], lhsT=sWo2[:], rhs=yo[:], start=True, stop=zero_bias
            )
            if not zero_bias:
                nc.tensor.matmul(
                    po[:], lhsT=sbo2[:], rhs=ones_r[:], start=False, stop=True
                )
            if j % OBW == 0:
                d2 = st_blk.setdefault("ob%d" % (j // OBW), {})
                d2["ob"] = macroP.tile([OUT, OBW * PB], F32, tag="ob", name="ob")
            ob = st_blk["ob%d" % (j // OBW)]["ob"]
            nc.vector.tensor_copy(
                out=ob[:, (j % OBW) * PB : (j % OBW + 1) * PB], in_=po[:]
            )
            if j % OBW == OBW - 1:
                nc.sync.dma_start(
                    out=outT[:, (j - OBW + 1) * PB : (j + 1) * PB], in_=ob[:]
                )
            del st_blk[j]

        for j in range(NPB + 2):
            if j >= 1 and j - 1 < NPB:
                p2_ln(j - 1)
            if j >= 2:
                p3_out(j - 2)
            if j < NPB:
                p1_front(j)

    nc.compile()
    return nc


def _prep_core(ef_b, snd_b, rcv_b, rn_b, pn_b, tau_b, q):
    import ml_dtypes

    lo = q * QP
    mask = (rcv_b >= lo) & (rcv_b < lo + QP)
    ed = np.nonzero(mask)[0]
    loc = (rcv_b[ed] - lo).astype(np.int64)
    order = np.argsort(loc, kind="stable")
    ed, loc = ed[order], loc[order]
    grp = loc // G
    cnts = np.bincount(grp, minlength=NG)
    assert cnts.max() <= EPG, f"group overflow: {cnts.max()} > {EPG}"
    gstart = np.concatenate([[0], np.cumsum(cnts)[:-1]])
    slot = grp * EPG + (np.arange(len(ed)) - gstart[grp])

    efp = np.zeros((NEP, EIN), np.float32)
    efp[slot] = ef_b[ed]
    # host pre-gather of sender/receiver features per edge slot
    sfp = np.zeros((NEP, F), np.float32)
    sfp[slot] = rn_b[snd_b[ed]]
    rfp = np.zeros((NEP, F), np.float32)
    rfp[slot] = pn_b[rcv_b[ed]]
    rrel = np.full(NEP, -1.0, np.float32)
    rrel[slot] = (loc - grp * G).astype(np.float32)
    # precomputed one-hot [slot -> segment] per group, laid out
    # [128 partitions, NG, 4 chunks, G] with slot = chunk*128 + partition
    ohm = (
        rrel.reshape(NG, 4, 128, 1) == np.arange(G, dtype=np.float32)
    ).astype(np.float32)
    ohm = ohm.transpose(2, 0, 1, 3).reshape(128, NG * 4 * G)

    cnt = np.bincount(loc, minlength=QP).astype(np.float32)
    inv = (1.0 / np.maximum(cnt, 1.0)).astype(np.float32)
    m01 = np.minimum(cnt, 1.0)

    pn_q = pn_b[lo : lo + QP]
    return {
        "efT": np.ascontiguousarray(efp.T.astype(ml_dtypes.bfloat16)),
        "sfT": np.ascontiguousarray(sfp.T.astype(ml_dtypes.bfloat16)),
        "rfT": np.ascontiguousarray(rfp.T.astype(ml_dtypes.bfloat16)),
        "ohM": np.ascontiguousarray(ohm).astype(ml_dtypes.bfloat16),
        "pnT": np.ascontiguousarray(pn_q.T).astype(ml_dtypes.bfloat16),
        "invQ": inv.reshape(1, QP),
        "m01Q": m01.reshape(1, QP).astype(ml_dtypes.bfloat16),
        "tau": tau_b.reshape(1, 1).astype(np.float32),
    }


def _prep_weights(i):
    w = {
        "We1": i["We1"], "be1": i["be1"].reshape(F, 1), "We2": i["We2"],
        "be2_4": np.tile(i["be2"].reshape(1, F), (1, 4)),
        "Wu1a": i["Wu1"][0:F], "Wu1b": i["Wu1"][F : 2 * F],
        "Wu1c": i["Wu1"][2 * F : 3 * F],
        "bu1": i["bu1"].reshape(F, 1), "Wu2": i["Wu2"],
        "bu2_4": np.tile(i["bu2"].reshape(1, F), (1, 4)),
        "Wp1n": i["Wp1"][0:F], "Wp1g": i["Wp1"][F : 2 * F],
        "bp1": i["bp1"].reshape(F, 1), "Wp2": i["Wp2"],
        "bp2_4": np.tile(i["bp2"].reshape(1, F), (1, 4)),
        "Wo1": i["Wo1"], "bo1": i["bo1"].reshape(F, 1), "Wo2": i["Wo2"],
        "bo2": i["bo2"].reshape(1, OUT),
    }
    for k in ("e", "u", "p"):
        C1, c1 = i[f"C{k}1"], i[f"c{k}1"]
        C2, c2 = i[f"C{k}2"], i[f"c{k}2"]
        w[f"C{k}1"] = C1.reshape(1, H)
        w[f"c{k}1"] = c1.reshape(H, 1)
        w[f"C{k}2a"] = np.ascontiguousarray(C2[:, 0:F])
        w[f"C{k}2b"] = np.ascontiguousarray(C2[:, F : 2 * F])
        w[f"c{k}2a1"] = (c2[0:F] + 1.0).reshape(F, 1)
        w[f"c{k}2b"] = c2[F : 2 * F].reshape(F, 1)
    import ml_dtypes

    bf16_keys = {"We1", "We2", "Wu1b", "Wu1c", "Wu2", "Wp1n", "Wp2", "Wo2",
                 "be2_4", "bu2_4", "bp2_4", "bo2"}
    return {
        k: np.ascontiguousarray(
            v, dtype=ml_dtypes.bfloat16 if k in bf16_keys else np.float32
        )
        for k, v in w.items()
    }


_NC_CACHE = {}


def _zero_bias(i):
    return all(
        float(np.abs(np.asarray(i[k])).max()) == 0.0
        for k in ("be2", "bu2", "bp2", "bo2")
    )


def build_in_maps(inputs):
    i = {k: np.asarray(v) for k, v in inputs.items()}
    w = _prep_weights(i)
    in_maps = []
    for core in range(8):
        b, q = core // NQ, core % NQ
        m = dict(w)
        m.update(
            _prep_core(
                i["edge_features"][b], i["senders"][b], i["receivers"][b],
                i["rnode_features"][b], i["pnode_features"][b], i["tau"][b], q
            )
        )
        in_maps.append(m)
    return in_maps


def get_nc(zero_bias=True):
    key = ("nc", bool(zero_bias))
    if key not in _NC_CACHE:
        _NC_CACHE[key] = _build_nc(bool(zero_bias))
    return _NC_CACHE[key]


def assemble(results):
    out = np.zeros((B, NPTOT, OUT), np.float32)
    for core in range(8):
        b, q = core // NQ, core % NQ
        out[b, q * QP : (q + 1) * QP, :] = results[core]["outT"].T
    return out


def kernel(**inputs):
    from concourse.bass_utils import run_bass_kernel_spmd

    i = {k: np.asarray(v) for k, v in inputs.items()}
    nc = get_nc(_zero_bias(i))
    in_maps = build_in_maps(i)
    res = run_bass_kernel_spmd(nc, in_maps, list(range(8)))
    return assemble(res.results)


if __name__ == "__main__":
    import reference

    inputs = reference.setup_inputs()
    out = kernel(**{k: np.asarray(v) for k, v in inputs.items()})
    print("out", out.shape, out.dtype)


# revision 34
# speedup vs baseline: 1.2499x; 1.2499x over previous
"""Trainium2 Bass kernel for the GNN decoder (message passing, cond-layernorm).

Sharding: 8 cores = (batch b in {0,1}) x (pnode quarter q in {0..3}).
Each core owns pnode rows [q*16384, (q+1)*16384) of its batch and every edge
whose receiver lands in that range.  Edges are receiver-sorted on the host and
packed into NG groups of G=104 consecutive segments with a fixed budget of
EPG=512 edge slots per group (padded; pad slots have one-hot row == 0 so they
contribute nothing).

Host-side prep pre-gathers sender/receiver features per edge slot (sfT/rfT,
bf16, streamed from DRAM) so the device loop needs no gpsimd gathers.  The
edge loop is software-pipelined across groups (stages S1..S5 emitted with
skews 0/1/1/2/3) so each engine's in-order queue always has ready work; the
pnode loop is pipelined the same way.  Segment sums accumulate via one-hot
matmuls into SBUF-resident aggregate tables consumed directly by the pnode
phase.  Cond-norm affines are folded into downstream weights on device once
per launch.  The scalar engine only ever runs {silu, identity, copy} (one act
table set); layernorm rsqrt runs on the vector engine via a quake-style
bitcast seed plus one Newton step.  MLP bias rank-1 matmuls are emitted only
if the corresponding host-side bias vectors are nonzero (they are
structurally zero in this model).
"""

import numpy as np

import concourse.bass as bass
import concourse.tile as tile
from concourse import bacc
from concourse import mybir
from concourse.masks import make_identity

F32 = mybir.dt.float32
BF16 = mybir.dt.bfloat16
I32 = mybir.dt.int32

B, NR, NPTOT, E, F, EIN, H, OUT = 2, 16384, 65536, 262144, 128, 4, 16, 4
EPS = 1e-6
NQ = 4                  # pnode quarters per batch
QP = NPTOT // NQ        # pnodes per core (16384)
G = 104                 # segments per group
EPG = 512               # edge slots per group
NG = (QP + G - 1) // G  # groups per core (158)
NEP = NG * EPG          # padded edge slots per core
PB = 512                # pnode block width
NPB = QP // PB          # pnode blocks per core (32)
M = 2                   # groups per macro DMA batch

AF = mybir.ActivationFunctionType
ALU = mybir.AluOpType


def _build_nc(zero_bias):
    nc = bacc.Bacc("TRN2", target_bir_lowering=False, debug=False)

    def inp(name, shape, dtype=F32):
        return nc.dram_tensor(name, shape, dtype, kind="ExternalInput")

    efT_d = inp("efT", [EIN, NEP], BF16)
    sfT_d = inp("sfT", [F, NEP], BF16)
    rfT_d = inp("rfT", [F, NEP], BF16)
    oh_d = inp("ohM", [128, NG * 4 * G], BF16)
    pnT_d = inp("pnT", [F, QP], BF16)
    tau_d = inp("tau", [1, 1])
    inv_d = inp("invQ", [1, QP])
    m01_d = inp("m01Q", [1, QP], BF16)

    We1 = inp("We1", [EIN, F], BF16)
    be1 = inp("be1", [F, 1])
    We2 = inp("We2", [F, F], BF16)
    be2_4 = inp("be2_4", [1, 4 * F], BF16)
    Wu1a = inp("Wu1a", [F, F])
    Wu1b = inp("Wu1b", [F, F], BF16)
    Wu1c = inp("Wu1c", [F, F], BF16)
    bu1 = inp("bu1", [F, 1])
    Wu2 = inp("Wu2", [F, F], BF16)
    bu2_4 = inp("bu2_4", [1, 4 * F], BF16)
    Wp1n = inp("Wp1n", [F, F], BF16)
    Wp1g = inp("Wp1g", [F, F])
    bp1 = inp("bp1", [F, 1])
    Wp2 = inp("Wp2", [F, F], BF16)
    bp2_4 = inp("bp2_4", [1, 4 * F], BF16)
    Wo1 = inp("Wo1", [F, F])
    bo1 = inp("bo1", [F, 1])
    Wo2 = inp("Wo2", [F, OUT], BF16)
    bo2 = inp("bo2", [1, OUT], BF16)
    # conditioning nets: e(dge embed), u(pdate), p(node).  r is dead code.
    cond_w = {}
    for k in ("e", "u", "p"):
        cond_w[k] = (
            inp(f"C{k}1", [1, H]),
            inp(f"c{k}1", [H, 1]),
            inp(f"C{k}2a", [H, F]),     # scale half of C2
            inp(f"C{k}2b", [H, F]),     # shift half of C2
            inp(f"c{k}2a1", [F, 1]),    # c2[:F] + 1.0
            inp(f"c{k}2b", [F, 1]),     # c2[F:]
        )

    outT = nc.dram_tensor("outT", [OUT, QP], F32, kind="ExternalOutput")

    from contextlib import ExitStack

    with tile.TileContext(nc) as tc, ExitStack() as ctx:
        singles = ctx.enter_context(tc.tile_pool(name="singles", bufs=1))
        macro = ctx.enter_context(tc.tile_pool(name="macro", bufs=4))
        macroP = ctx.enter_context(tc.tile_pool(name="macroP", bufs=2))
        work = ctx.enter_context(tc.tile_pool(name="work", bufs=2))
        work3 = ctx.enter_context(tc.tile_pool(name="work3", bufs=3))
        small = ctx.enter_context(tc.tile_pool(name="small", bufs=4))
        psB = ctx.enter_context(tc.tile_pool(name="psB", bufs=2, space="PSUM"))
        psA = ctx.enter_context(tc.tile_pool(name="psA", bufs=2, space="PSUM"))
        psT = ctx.enter_context(tc.tile_pool(name="psT", bufs=2, space="PSUM"))
        psS = ctx.enter_context(tc.tile_pool(name="psS", bufs=2, space="PSUM"))

        # ---------- constants & resident tables ----------
        def load(name, dram, shape, dtype=F32):
            t = singles.tile(shape, dtype, tag=name)
            nc.sync.dma_start(out=t[:], in_=dram[:])
            return t

        pnT = load("pnT", pnT_d, [F, QP], BF16)
        sWe1 = load("We1", We1, [EIN, F], BF16)
        sbe1 = load("be1", be1, [F, 1])
        sWe2 = load("We2", We2, [F, F], BF16)
        sWu1a = load("Wu1a", Wu1a, [F, F])
        sWu1b = load("Wu1b", Wu1b, [F, F], BF16)
        sWu1c = load("Wu1c", Wu1c, [F, F], BF16)
        sbu1 = load("bu1", bu1, [F, 1])
        sWu2 = load("Wu2", Wu2, [F, F], BF16)
        sWp1n = load("Wp1n", Wp1n, [F, F], BF16)
        sWp1g = load("Wp1g", Wp1g, [F, F])
        sbp1 = load("bp1", bp1, [F, 1])
        sWp2 = load("Wp2", Wp2, [F, F], BF16)
        sWo1 = load("Wo1", Wo1, [F, F])
        sbo1 = load("bo1", bo1, [F, 1])
        sWo2 = load("Wo2", Wo2, [F, OUT], BF16)
        stau = load("tau", tau_d, [1, 1])
        if not zero_bias:
            sbe2_4 = load("be2_4", be2_4, [1, 4 * F], BF16)
            sbu2_4 = load("bu2_4", bu2_4, [1, 4 * F], BF16)
            sbp2_4 = load("bp2_4", bp2_4, [1, 4 * F], BF16)
            sbo2 = load("bo2", bo2, [1, OUT], BF16)

        ident = singles.tile([128, 128], F32, tag="ident")
        make_identity(nc, ident[:])
        ident16 = singles.tile([128, 128], BF16, tag="ident16")
        nc.vector.tensor_copy(out=ident16[:], in_=ident[:])
        ones_r = singles.tile([1, PB], BF16, tag="ones_r")
        nc.vector.memset(ones_r[:], 1.0)
        ones_r32 = singles.tile([1, 128], F32, tag="ones_r32")
        nc.vector.memset(ones_r32[:], 1.0)

        # SBUF-resident aggregation tables (written per group, read per block)
        aggS1 = singles.tile([128, NG * G], BF16, tag="aggS1")
        aggS2 = singles.tile([128, NG * G], BF16, tag="aggS2")

        magic = singles.tile([128, 8], I32, tag="magic")
        nc.vector.memset(magic[:], 0x5F3759DF)

        # ---------- conditioning nets (tau -> scale/shift) + weight folds ----------
        cvec = {}
        for k in ("e", "u", "p"):
            C1, c1, C2a, C2b, c2a1, c2b = cond_w[k]
            sC1 = load(f"C{k}1", C1, [1, H])
            sc1 = load(f"c{k}1", c1, [H, 1])
            sC2a = load(f"C{k}2a", C2a, [H, F])
            sC2b = load(f"C{k}2b", C2b, [H, F])
            sc2a1 = load(f"c{k}2a1", c2a1, [F, 1])
            sc2b = load(f"c{k}2b", c2b, [F, 1])

            ph = psS.tile([H, 1], F32, tag="pS")
            nc.tensor.matmul(ph[:], lhsT=sC1[:], rhs=stau[:], start=True, stop=True)
            hs = small.tile([H, 1], F32, tag=f"hs{k}")
            nc.scalar.activation(hs[:], ph[:], AF.Silu, bias=sc1[:], scale=1.0)

            pscale = psS.tile([F, 1], F32, tag="pS")
            nc.tensor.matmul(pscale[:], lhsT=sC2a[:], rhs=hs[:], start=True, stop=True)
            s1p = singles.tile([F, 1], F32, tag=f"s1p{k}")
            nc.vector.tensor_scalar(
                out=s1p[:], in0=pscale[:], scalar1=sc2a1[:], scalar2=None, op0=ALU.add
            )
            pshift = psS.tile([F, 1], F32, tag="pS")
            nc.tensor.matmul(pshift[:], lhsT=sC2b[:], rhs=hs[:], start=True, stop=True)
            shift = singles.tile([F, 1], F32, tag=f"shift{k}")
            nc.vector.tensor_scalar(
                out=shift[:], in0=pshift[:], scalar1=sc2b[:], scalar2=None, op0=ALU.add
            )
            cvec[k] = (s1p, shift)

        s1pe, shifte = cvec["e"]
        s1pu, shiftu = cvec["u"]
        s1pp, shiftp = cvec["p"]

        # fold cond-norm affines into downstream weights
        fWu1a = singles.tile([F, F], BF16, tag="fWu1a")
        nc.vector.tensor_tensor(
            out=fWu1a[:], in0=sWu1a[:], in1=s1pe[:].to_broadcast([F, F]), op=ALU.mult
        )
        pbu1 = psS.tile([F, 1], F32, tag="pS")
        nc.tensor.matmul(pbu1[:], lhsT=sWu1a[:], rhs=shifte[:], start=True, stop=True)
        fbu1 = singles.tile([F, 1], F32, tag="fbu1")
        nc.vector.tensor_scalar(
            out=fbu1[:], in0=pbu1[:], scalar1=sbu1[:], scalar2=None, op0=ALU.add
        )

        fWp1ge = singles.tile([F, F], BF16, tag="fWp1ge")
        nc.vector.tensor_tensor(
            out=fWp1ge[:], in0=sWp1g[:], in1=s1pe[:].to_broadcast([F, F]), op=ALU.mult
        )
        fWp1gu = singles.tile([F, F], BF16, tag="fWp1gu")
        nc.vector.tensor_tensor(
            out=fWp1gu[:], in0=sWp1g[:], in1=s1pu[:].to_broadcast([F, F]), op=ALU.mult
        )
        shifteu = small.tile([F, 1], F32, tag="shifteu")
        nc.vector.tensor_tensor(
            out=shifteu[:], in0=shifte[:], in1=shiftu[:], op=ALU.add
        )
        pbpe = psS.tile([1, F], F32, tag="pS")
        nc.tensor.matmul(pbpe[:], lhsT=shifteu[:], rhs=sWp1g[:], start=True, stop=True)
        bpe_row = singles.tile([1, F], BF16, tag="bpe_row")
        nc.vector.tensor_copy(out=bpe_row[:], in_=pbpe[:])

        fWo1 = singles.tile([F, F], BF16, tag="fWo1")
        nc.vector.tensor_tensor(
            out=fWo1[:], in0=sWo1[:], in1=s1pp[:].to_broadcast([F, F]), op=ALU.mult
        )
        sWo116 = singles.tile([F, F], BF16, tag="sWo116")
        nc.vector.tensor_copy(out=sWo116[:], in_=sWo1[:])
        pbo1 = psS.tile([F, 1], F32, tag="pS")
        nc.tensor.matmul(pbo1[:], lhsT=sWo1[:], rhs=shiftp[:], start=True, stop=True)
        fbo1 = singles.tile([F, 1], F32, tag="fbo1")
        nc.vector.tensor_scalar(
            out=fbo1[:], in0=pbo1[:], scalar1=sbo1[:], scalar2=None, op0=ALU.add
        )

        # second MLP layer: 4 chunk matmuls (+ bias rank-1s only if nonzero)
        def emit_l2(psum4, ysrc, W, bias4):
            for c in range(4):
                nc.tensor.matmul(
                    psum4[:, c, :],
                    lhsT=ysrc[:, c * 128 : (c + 1) * 128],
                    rhs=W[:],
                    start=True,
                    stop=zero_bias,
                )
                if not zero_bias:
                    nc.tensor.matmul(
                        psum4[:, c, :], lhsT=ones_r[:, 0:128],
                        rhs=bias4[:, 0:128], start=False, stop=True,
                    )

        # layer-norm split: per-chunk bn stats into a shared mv tile, one
        # batched rsqrt chain (bitcast seed + 1 NR) for up to 2 LNs, then
        # per-chunk applies (3 on scalar via Identity, 1 on vector).
        magic8 = magic  # [128, 8] int32 0x5f3759df

        def ln_stats(psum4, mv8, half, tag):
            st = small.tile([128, 4, 6], F32, tag=f"st{tag}", name="st")
            for c in range(4):
                nc.vector.bn_stats(out=st[:, c, :], in_=psum4[:, c, :])
            for c in range(4):
                nc.vector.bn_aggr(out=mv8[:, half * 4 + c, :], in_=st[:, c, :])

        def ln_rsqrt(mv8, W, tag):
            vpe = small.tile([128, W], F32, tag=f"vp{tag}", name="vpe")
            nc.vector.tensor_scalar(
                out=vpe[:], in0=mv8[:, 0:W, 1], scalar1=EPS, scalar2=None,
                op0=ALU.add,
            )
            ish = small.tile([128, W], I32, tag=f"is{tag}", name="ish")
            nc.vector.tensor_scalar(
                out=ish[:], in0=vpe[:].bitcast(I32), scalar1=1, scalar2=None,
                op0=ALU.arith_shift_right,
            )
            y0i = small.tile([128, W], I32, tag=f"y0{tag}", name="y0i")
            nc.vector.tensor_tensor(
                out=y0i[:], in0=magic8[:, 0:W], in1=ish[:], op=ALU.subtract
            )
            y0 = y0i[:].bitcast(F32)
            y0sq = small.tile([128, W], F32, tag=f"yq{tag}", name="y0sq")
            nc.vector.tensor_tensor(out=y0sq[:], in0=y0, in1=y0, op=ALU.mult)
            th = small.tile([128, W], F32, tag=f"th{tag}", name="th")
            nc.vector.scalar_tensor_tensor(
                out=th[:], in0=y0sq[:], scalar=-0.5, in1=vpe[:],
                op0=ALU.mult, op1=ALU.mult,
            )
            rstd = small.tile([128, W], F32, tag=f"rs{tag}", name="rstd")
            nc.vector.scalar_tensor_tensor(
                out=rstd[:], in0=th[:], scalar=1.5, in1=y0,
                op0=ALU.add, op1=ALU.mult,
            )
            nmr = small.tile([128, W], F32, tag=f"nm{tag}", name="nmr")
            nc.vector.scalar_tensor_tensor(
                out=nmr[:], in0=mv8[:, 0:W, 0], scalar=-1.0, in1=rstd[:],
                op0=ALU.mult, op1=ALU.mult,
            )
            return rstd, nmr

        def ln_apply(psum4, out4, rstd, nmr, mv8, half):
            for c in range(4):
                w = half * 4 + c
                if c % 2 == 1:
                    nc.vector.tensor_scalar(
                        out=out4[:, c, :],
                        in0=psum4[:, c, :],
                        scalar1=mv8[:, w, 0:1],
                        scalar2=rstd[:, w : w + 1],
                        op0=ALU.subtract,
                        op1=ALU.mult,
                    )
                else:
                    nc.scalar.activation(
                        out4[:, c, :], psum4[:, c, :], AF.Identity,
                        bias=nmr[:, w : w + 1], scale=rstd[:, w : w + 1],
                    )

        # transpose [128, 4, 128] sbuf bf16 -> [128, 512] sbuf bf16 (feat, slot)
        def transp(ln4, outT_t):
            ptr = psT.tile([128, 4, 128], BF16, tag="pT", name="ptr")
            for c in range(4):
                nc.tensor.transpose(ptr[:, c, :], ln4[:, c, :], ident16[:])
            for c in range(4):
                if c % 2 == 0:
                    nc.vector.tensor_copy(
                        out=outT_t[:, c * 128 : (c + 1) * 128], in_=ptr[:, c, :]
                    )
                else:
                    nc.scalar.copy(
                        out=outT_t[:, c * 128 : (c + 1) * 128], in_=ptr[:, c, :]
                    )

        # ---------- edge phase (software-pipelined across groups) ----------
        st_macro = {}   # macro index -> dict of macro tiles
        st_grp = {}     # group index -> dict of per-group tiles

        def macro_load(m):
            g0 = m * M
            gn = min(M, NG - g0)
            ew = gn * EPG
            t = {}
            t["ef"] = macro.tile([EIN, M * EPG], BF16, tag="efM", name="efM")
            nc.sync.dma_start(
                out=t["ef"][:, 0:ew], in_=efT_d[:, g0 * EPG : g0 * EPG + ew]
            )
            t["sf"] = macro.tile([F, M * EPG], BF16, tag="sfM", name="sfM")
            nc.sync.dma_start(
                out=t["sf"][:, 0:ew], in_=sfT_d[:, g0 * EPG : g0 * EPG + ew]
            )
            t["rf"] = macro.tile([F, M * EPG], BF16, tag="rfM", name="rfM")
            nc.sync.dma_start(
                out=t["rf"][:, 0:ew], in_=rfT_d[:, g0 * EPG : g0 * EPG + ew]
            )
            t["oh"] = macro.tile([128, M * 4 * G], BF16, tag="ohM", name="ohM")
            nc.sync.dma_start(
                out=t["oh"][:, 0 : gn * 4 * G],
                in_=oh_d[:, g0 * 4 * G : (g0 + gn) * 4 * G],
            )
            st_macro[m] = t

        def s1_embed(g):
            mt = st_macro[g // M]
            esl = slice((g % M) * EPG, (g % M + 1) * EPG)
            d = st_grp.setdefault(g, {})
            d["esl"] = esl
            pz1 = psB.tile([128, EPG], F32, tag="pB")
            nc.tensor.matmul(
                pz1[:], lhsT=sWe1[:], rhs=mt["ef"][:, esl], start=True, stop=True
            )
            y1 = work.tile([128, EPG], BF16, tag="y1")
            nc.scalar.activation(y1[:], pz1[:], AF.Silu, bias=sbe1[:], scale=1.0)
            pz2 = psA.tile([128, 4, 128], F32, tag="pA")
            emit_l2(pz2, y1, sWe2, None if zero_bias else sbe2_4)
            d["pz2"] = pz2

        def ln_driver(i):
            jobs = []
            if 1 <= i and i - 1 < NG:
                jobs.append(("e", i - 1))
            if 2 <= i and i - 2 < NG:
                jobs.append(("u", i - 2))
            if not jobs:
                return
            mv8 = small.tile([128, 8, 2], F32, tag="mv8", name="mv8")
            for h, (kind, g) in enumerate(jobs):
                d = st_grp[g]
                ln_stats(d["pz2" if kind == "e" else "pu2"], mv8, h, kind)
            rstd, nmr = ln_rsqrt(mv8, 4 * len(jobs), "eu")
            for h, (kind, g) in enumerate(jobs):
                d = st_grp[g]
                if kind == "e":
                    ln1 = work3.tile([128, 4, 128], BF16, tag="ln1", name="ln1")
                    ln_apply(d["pz2"], ln1, rstd, nmr, mv8, h)
                    d["ln1"] = ln1
                    del d["pz2"]
                    ln1T = work.tile([128, EPG], BF16, tag="ln1T", name="ln1T")
                    transp(ln1, ln1T)
                    d["ln1T"] = ln1T
                else:
                    ln2 = work.tile([128, 4, 128], BF16, tag="ln2", name="ln2")
                    ln_apply(d["pu2"], ln2, rstd, nmr, mv8, h)
                    d["ln2"] = ln2
                    del d["pu2"]

        def s3_update(g):
            d = st_grp[g]
            mt = st_macro[g // M]
            esl = d["esl"]
            pu1 = psB.tile([128, EPG], F32, tag="pB")
            nc.tensor.matmul(
                pu1[:], lhsT=fWu1a[:], rhs=d["ln1T"][:], start=True, stop=False
            )
            nc.tensor.matmul(
                pu1[:], lhsT=sWu1b[:], rhs=mt["sf"][:, esl], start=False, stop=False
            )
            nc.tensor.matmul(
                pu1[:], lhsT=sWu1c[:], rhs=mt["rf"][:, esl], start=False, stop=True
            )
            del d["ln1T"]
            yu = work.tile([128, EPG], BF16, tag="yu")
            nc.scalar.activation(yu[:], pu1[:], AF.Silu, bias=fbu1[:], scale=1.0)
            pu2 = psA.tile([128, 4, 128], F32, tag="pA")
            emit_l2(pu2, yu, sWu2, None if zero_bias else sbu2_4)
            d["pu2"] = pu2

        def s5_agg(g):
            d = st_grp[g]
            mt = st_macro[g // M]
            gm = g % M
            Sps = psS.tile([128, 2 * G], F32, tag="pS")
            for c in range(4):
                nc.tensor.matmul(
                    Sps[:, 0:G],
                    lhsT=d["ln1"][:, c, :],
                    rhs=mt["oh"][:, (gm * 4 + c) * G : (gm * 4 + c + 1) * G],
                    start=(c == 0),
                    stop=(c == 3),
                )
            for c in range(4):
                nc.tensor.matmul(
                    Sps[:, G : 2 * G],
                    lhsT=d["ln2"][:, c, :],
                    rhs=mt["oh"][:, (gm * 4 + c) * G : (gm * 4 + c + 1) * G],
                    start=(c == 0),
                    stop=(c == 3),
                )
            nc.vector.tensor_copy(
                out=aggS1[:, g * G : (g + 1) * G], in_=Sps[:, 0:G]
            )
            nc.scalar.copy(
                out=aggS2[:, g * G : (g + 1) * G], in_=Sps[:, G : 2 * G]
            )
            del st_grp[g]

        NMAC = (NG + M - 1) // M
        macro_load(0)
        for i in range(NG + 3):
            ln_driver(i)
            if i >= 3:
                s5_agg(i - 3)
            if i < NG:
                if i % M == 0 and (i // M) + 1 < NMAC:
                    macro_load(i // M + 1)
                s1_embed(i)
            if i >= 1 and i - 1 < NG:
                s3_update(i - 1)

        # ---------- pnode phase (software-pipelined across blocks) ----------
        OBW = 4  # blocks per staging window
        st_blk = {}

        def win_load(wi):
            w = {}
            w["inv"] = macroP.tile([1, OBW * PB], F32, tag="invW", name="invW")
            nc.sync.dma_start(
                out=w["inv"][:], in_=inv_d[:, wi * OBW * PB : (wi + 1) * OBW * PB]
            )
            w["m01"] = macroP.tile([1, OBW * PB], BF16, tag="m01W", name="m01W")
            nc.sync.dma_start(
                out=w["m01"][:], in_=m01_d[:, wi * OBW * PB : (wi + 1) * OBW * PB]
            )
            st_blk["w%d" % wi] = w

        def p1_front(j):
            d = st_blk.setdefault(j, {})
            sl = slice(j * PB, (j + 1) * PB)
            d["sl"] = sl
            if j % OBW == 0:
                if j == 0:
                    win_load(0)
                if j // OBW + 1 < NPB // OBW:
                    win_load(j // OBW + 1)
            w = st_blk["w%d" % (j // OBW)]
            wsl = slice((j % OBW) * PB, (j % OBW + 1) * PB)

            pinv = psT.tile([128, PB], F32, tag="pT")
            nc.tensor.matmul(
                pinv[:], lhsT=ones_r32[:], rhs=w["inv"][:, wsl],
                start=True, stop=True,
            )
            invb = work.tile([128, PB], F32, tag="invb")
            nc.scalar.copy(out=invb[:], in_=pinv[:])

            pA = psB.tile([128, PB], F32, tag="pB")
            nc.tensor.matmul(
                pA[:], lhsT=fWp1ge[:], rhs=aggS1[:, sl], start=True, stop=False
            )
            nc.tensor.matmul(
                pA[:], lhsT=fWp1gu[:], rhs=aggS2[:, sl], start=False, stop=True
            )
            tA = work.tile([128, PB], BF16, tag="tA")
            nc.vector.tensor_tensor(out=tA[:], in0=pA[:], in1=invb[:], op=ALU.mult)

            pzp = psB.tile([128, PB], F32, tag="pB")
            nc.tensor.matmul(
                pzp[:], lhsT=sWp1n[:], rhs=pnT[:, sl], start=True, stop=False
            )
            nc.tensor.matmul(
                pzp[:], lhsT=ident16[:], rhs=tA[:], start=False, stop=False
            )
            nc.tensor.matmul(
                pzp[:], lhsT=bpe_row[:], rhs=w["m01"][:, wsl],
                start=False, stop=True,
            )
            yp = work.tile([128, PB], BF16, tag="yp")
            nc.scalar.activation(yp[:], pzp[:], AF.Silu, bias=sbp1[:], scale=1.0)

            pp2 = psA.tile([128, 4, 128], F32, tag="pA")
            emit_l2(pp2, yp, sWp2, None if zero_bias else sbp2_4)
            d["pp2"] = pp2

        def p2_ln(j):
            d = st_blk[j]
            mv8 = small.tile([128, 8, 2], F32, tag="mv8", name="mv8")
            ln_stats(d["pp2"], mv8, 0, "p")
            rstd, nmr = ln_rsqrt(mv8, 4, "p")
            lnp = work3.tile([128, 4, 128], BF16, tag="ln1", name="lnp")
            ln_apply(d["pp2"], lnp, rstd, nmr, mv8, 0)
            del d["pp2"]
            lnpT = work.tile([128, PB], BF16, tag="ln1T", name="lnpT")
            transp(lnp, lnpT)
            d["lnpT"] = lnpT

        def p3_out(j):
            d = st_blk[j]
            sl = d["sl"]
            lnpT = d["lnpT"]
            pzo = psB.tile([128, PB], F32, tag="pB")
            nc.tensor.matmul(
                pzo[:], lhsT=fWo1[:], rhs=lnpT[:], start=True, stop=False
            )
            nc.tensor.matmul(
                pzo[:], lhsT=sWo116[:], rhs=pnT[:, sl], start=False, stop=True
            )
            yo = work.tile([128, PB], BF16, tag="yo")
            nc.scalar.activation(yo[:], pzo[:], AF.Silu, bias=fbo1[:], scale=1.0)

            po = psS.tile([OUT, PB], F32, tag="pS")
            nc.tensor.matmul(
                po[:], lhsT=sWo2[:], rhs=yo[:], start=True, stop=zero_bias
            )
            if not zero_bias:
                nc.tensor.matmul(
                    po[:], lhsT=sbo2[:], rhs=ones_r[:], start=False, stop=True
                )
            if j % OBW == 0:
                d2 = st_blk.setdefault("ob%d" % (j // OBW), {})
                d2["ob"] = macroP.tile([OUT, OBW * PB], F32, tag="ob", name="ob")
            ob = st_blk["ob%d" % (j // OBW)]["ob"]
            nc.vector.tensor_copy(
                out=ob[:, (j % OBW) * PB : (j % OBW + 1) * PB], in_=po[:]
            )
            if j % OBW == OBW - 1:
                nc.sync.dma_start(
                    out=outT[:, (j - OBW + 1) * PB : (j + 1) * PB], in_=ob[:]
                )
            del st_blk[j]

        for j in range(NPB + 2):
            if j >= 1 and j - 1 < NPB:
                p2_ln(j - 1)
            if j >= 2:
                p3_out(j - 2)
            if j < NPB:
                p1_front(j)

    nc.compile()
    return nc


def _prep_core(ef_b, snd_b, rcv_b, rn_b, pn_b, tau_b, q):
    import ml_dtypes

    lo = q * QP
    mask = (rcv_b >= lo) & (rcv_b < lo + QP)
    ed = np.nonzero(mask)[0]
    loc = (rcv_b[ed] - lo).astype(np.int64)
    order = np.argsort(loc, kind="stable")
    ed, loc = ed[order], loc[order]
    grp = loc // G
    cnts = np.bincount(grp, minlength=NG)
    assert cnts.max() <= EPG, f"group overflow: {cnts.max()} > {EPG}"
    gstart = np.concatenate([[0], np.cumsum(cnts)[:-1]])
    slot = grp * EPG + (np.arange(len(ed)) - gstart[grp])

    efp = np.zeros((NEP, EIN), np.float32)
    efp[slot] = ef_b[ed]
    # host pre-gather of sender/receiver features per edge slot
    sfp = np.zeros((NEP, F), np.float32)
    sfp[slot] = rn_b[snd_b[ed]]
    rfp = np.zeros((NEP, F), np.float32)
    rfp[slot] = pn_b[rcv_b[ed]]
    rrel = np.full(NEP, -1.0, np.float32)
    rrel[slot] = (loc - grp * G).astype(np.float32)
    # precomputed one-hot [slot -> segment] per group, laid out
    # [128 partitions, NG, 4 chunks, G] with slot = chunk*128 + partition
    ohm = (
        rrel.reshape(NG, 4, 128, 1) == np.arange(G, dtype=np.float32)
    ).astype(np.float32)
    ohm = ohm.transpose(2, 0, 1, 3).reshape(128, NG * 4 * G)

    cnt = np.bincount(loc, minlength=QP).astype(np.float32)
    inv = (1.0 / np.maximum(cnt, 1.0)).astype(np.float32)
    m01 = np.minimum(cnt, 1.0)

    pn_q = pn_b[lo : lo + QP]
    return {
        "efT": np.ascontiguousarray(efp.T.astype(ml_dtypes.bfloat16)),
        "sfT": np.ascontiguousarray(sfp.T.astype(ml_dtypes.bfloat16)),
        "rfT": np.ascontiguousarray(rfp.T.astype(ml_dtypes.bfloat16)),
        "ohM": np.ascontiguousarray(ohm).astype(ml_dtypes.bfloat16),
        "pnT": np.ascontiguousarray(pn_q.T).astype(ml_dtypes.bfloat16),
        "invQ": inv.reshape(1, QP),
        "m01Q": m01.reshape(1, QP).astype(ml_dtypes.bfloat16),
        "tau": tau_b.reshape(1, 1).astype(np.float32),
    }


def _prep_weights(i):
    w = {
        "We1": i["We1"], "be1": i["be1"].reshape(F, 1), "We2": i["We2"],
        "be2_4": np.tile(i["be2"].reshape(1, F), (1, 4)),
        "Wu1a": i["Wu1"][0:F], "Wu1b": i["Wu1"][F : 2 * F],
        "Wu1c": i["Wu1"][2 * F : 3 * F],
        "bu1": i["bu1"].reshape(F, 1), "Wu2": i["Wu2"],
        "bu2_4": np.tile(i["bu2"].reshape(1, F), (1, 4)),
        "Wp1n": i["Wp1"][0:F], "Wp1g": i["Wp1"][F : 2 * F],
        "bp1": i["bp1"].reshape(F, 1), "Wp2": i["Wp2"],
        "bp2_4": np.tile(i["bp2"].reshape(1, F), (1, 4)),
        "Wo1": i["Wo1"], "bo1": i["bo1"].reshape(F, 1), "Wo2": i["Wo2"],
        "bo2": i["bo2"].reshape(1, OUT),
    }
    for k in ("e", "u", "p"):
        C1, c1 = i[f"C{k}1"], i[f"c{k}1"]
        C2, c2 = i[f"C{k}2"], i[f"c{k}2"]
        w[f"C{k}1"] = C1.reshape(1, H)
        w[f"c{k}1"] = c1.reshape(H, 1)
        w[f"C{k}2a"] = np.ascontiguousarray(C2[:, 0:F])
        w[f"C{k}2b"] = np.ascontiguousarray(C2[:, F : 2 * F])
        w[f"c{k}2a1"] = (c2[0:F] + 1.0).reshape(F, 1)
        w[f"c{k}2b"] = c2[F : 2 * F].reshape(F, 1)
    import ml_dtypes

    bf16_keys = {"We1", "We2", "Wu1b", "Wu1c", "Wu2", "Wp1n", "Wp2", "Wo2",
                 "be2_4", "bu2_4", "bp2_4", "bo2"}
    return {
        k: np.ascontiguousarray(
            v, dtype=ml_dtypes.bfloat16 if k in bf16_keys else np.float32
        )
        for k, v in w.items()
    }


_NC_CACHE = {}


def _zero_bias(i):
    return all(
        float(np.abs(np.asarray(i[k])).max()) == 0.0
        for k in ("be2", "bu2", "bp2", "bo2")
    )


def build_in_maps(inputs):
    i = {k: np.asarray(v) for k, v in inputs.items()}
    w = _prep_weights(i)
    in_maps = []
    for core in range(8):
        b, q = core // NQ, core % NQ
        m = dict(w)
        m.update(
            _prep_core(
                i["edge_features"][b], i["senders"][b], i["receivers"][b],
                i["rnode_features"][b], i["pnode_features"][b], i["tau"][b], q
            )
        )
        in_maps.append(m)
    return in_maps


def get_nc(zero_bias=True):
    key = ("nc", bool(zero_bias))
    if key not in _NC_CACHE:
        _NC_CACHE[key] = _build_nc(bool(zero_bias))
    return _NC_CACHE[key]


def assemble(results):
    out = np.zeros((B, NPTOT, OUT), np.float32)
    for core in range(8):
        b, q = core // NQ, core % NQ
        out[b, q * QP : (q + 1) * QP, :] = results[core]["outT"].T
    return out


def kernel(**inputs):
    from concourse.bass_utils import run_bass_kernel_spmd

    i = {k: np.asarray(v) for k, v in inputs.items()}
    nc = get_nc(_zero_bias(i))
    in_maps = build_in_maps(i)
    res = run_bass_kernel_spmd(nc, in_maps, list(range(8)))
    return assemble(res.results)


if __name__ == "__main__":
    import reference

    inputs = reference.setup_inputs()
    out = kernel(**{k: np.asarray(v) for k, v in inputs.items()})
    print("out", out.shape, out.dtype)


# revision 35
# speedup vs baseline: 1.6320x; 1.3057x over previous
"""Trainium2 Bass kernel for the GNN decoder (message passing, cond-layernorm).

Sharding: 8 cores = (batch b in {0,1}) x (pnode quarter q in {0..3}).
Each core owns pnode rows [q*16384, (q+1)*16384) of its batch and every edge
whose receiver lands in that range.  Edges are receiver-sorted on the host and
packed into NG groups of G=104 consecutive segments with a fixed budget of
EPG=512 edge slots per group (padded; pad slots have one-hot row == 0 so they
contribute nothing).

Host-side prep pre-gathers sender/receiver features per edge slot (sfT/rfT,
bf16, streamed from DRAM) so the device loop needs no gpsimd gathers.  The
edge loop is software-pipelined across groups (stages S1..S5 emitted with
skews 0/1/1/2/3) so each engine's in-order queue always has ready work; the
pnode loop is pipelined the same way.  Segment sums accumulate via one-hot
matmuls into SBUF-resident aggregate tables consumed directly by the pnode
phase.  Cond-norm affines are folded into downstream weights on device once
per launch.  The scalar engine only ever runs {silu, identity, copy} (one act
table set); layernorm rsqrt runs on the vector engine via a quake-style
bitcast seed plus one Newton step.  MLP bias rank-1 matmuls are emitted only
if the corresponding host-side bias vectors are nonzero (they are
structurally zero in this model).
"""

import numpy as np

import concourse.bass as bass
import concourse.tile as tile
from concourse import bacc
from concourse import mybir
from concourse.masks import make_identity

F32 = mybir.dt.float32
BF16 = mybir.dt.bfloat16
I32 = mybir.dt.int32

B, NR, NPTOT, E, F, EIN, H, OUT = 2, 16384, 65536, 262144, 128, 4, 16, 4
EPS = 1e-6
NQ = 4                  # pnode quarters per batch
QP = NPTOT // NQ        # pnodes per core (16384)
G = 104                 # segments per group
EPG = 512               # edge slots per group
NG = (QP + G - 1) // G  # groups per core (158)
NEP = NG * EPG          # padded edge slots per core
PB = 512                # pnode block width
NPB = QP // PB          # pnode blocks per core (32)
M = 2                   # groups per macro DMA batch

AF = mybir.ActivationFunctionType
ALU = mybir.AluOpType


def _build_nc(zero_bias):
    nc = bacc.Bacc("TRN2", target_bir_lowering=False, debug=False)

    def inp(name, shape, dtype=F32):
        return nc.dram_tensor(name, shape, dtype, kind="ExternalInput")

    efT_d = inp("efT", [EIN, NEP], BF16)
    sfT_d = inp("sfT", [F, NEP], BF16)
    rfT_d = inp("rfT", [F, NEP], BF16)
    oh_d = inp("ohM", [128, NG * 4 * G], BF16)
    pnT_d = inp("pnT", [F, QP], BF16)
    tau_d = inp("tau", [1, 1])
    inv_d = inp("invQ", [1, QP])
    m01_d = inp("m01Q", [1, QP], BF16)

    We1 = inp("We1", [EIN, F], BF16)
    be1 = inp("be1", [F, 1])
    We2 = inp("We2", [F, F], BF16)
    be2_4 = inp("be2_4", [1, 4 * F], BF16)
    Wu1a = inp("Wu1a", [F, F])
    Wu1b = inp("Wu1b", [F, F], BF16)
    Wu1c = inp("Wu1c", [F, F], BF16)
    bu1 = inp("bu1", [F, 1])
    Wu2 = inp("Wu2", [F, F], BF16)
    bu2_4 = inp("bu2_4", [1, 4 * F], BF16)
    Wp1n = inp("Wp1n", [F, F], BF16)
    Wp1g = inp("Wp1g", [F, F])
    bp1 = inp("bp1", [F, 1])
    Wp2 = inp("Wp2", [F, F], BF16)
    bp2_4 = inp("bp2_4", [1, 4 * F], BF16)
    Wo1 = inp("Wo1", [F, F])
    bo1 = inp("bo1", [F, 1])
    Wo2 = inp("Wo2", [F, OUT], BF16)
    bo2 = inp("bo2", [1, OUT], BF16)
    # conditioning nets: e(dge embed), u(pdate), p(node).  r is dead code.
    cond_w = {}
    for k in ("e", "u", "p"):
        cond_w[k] = (
            inp(f"C{k}1", [1, H]),
            inp(f"c{k}1", [H, 1]),
            inp(f"C{k}2a", [H, F]),     # scale half of C2
            inp(f"C{k}2b", [H, F]),     # shift half of C2
            inp(f"c{k}2a1", [F, 1]),    # c2[:F] + 1.0
            inp(f"c{k}2b", [F, 1]),     # c2[F:]
        )

    outT = nc.dram_tensor("outT", [OUT, QP], F32, kind="ExternalOutput")

    from contextlib import ExitStack

    with tile.TileContext(nc) as tc, ExitStack() as ctx:
        singles = ctx.enter_context(tc.tile_pool(name="singles", bufs=1))
        macro = ctx.enter_context(tc.tile_pool(name="macro", bufs=4))
        macroP = ctx.enter_context(tc.tile_pool(name="macroP", bufs=2))
        work = ctx.enter_context(tc.tile_pool(name="work", bufs=2))
        work3 = ctx.enter_context(tc.tile_pool(name="work3", bufs=3))
        small = ctx.enter_context(tc.tile_pool(name="small", bufs=4))
        psB = ctx.enter_context(tc.tile_pool(name="psB", bufs=2, space="PSUM"))
        psA = ctx.enter_context(tc.tile_pool(name="psA", bufs=2, space="PSUM"))
        psT = ctx.enter_context(tc.tile_pool(name="psT", bufs=2, space="PSUM"))
        psS = ctx.enter_context(tc.tile_pool(name="psS", bufs=2, space="PSUM"))

        # ---------- constants & resident tables ----------
        def load(name, dram, shape, dtype=F32):
            t = singles.tile(shape, dtype, tag=name)
            nc.sync.dma_start(out=t[:], in_=dram[:])
            return t

        pnT = load("pnT", pnT_d, [F, QP], BF16)
        sWe1 = load("We1", We1, [EIN, F], BF16)
        sbe1 = load("be1", be1, [F, 1])
        sWe2 = load("We2", We2, [F, F], BF16)
        sWu1a = load("Wu1a", Wu1a, [F, F])
        sWu1b = load("Wu1b", Wu1b, [F, F], BF16)
        sWu1c = load("Wu1c", Wu1c, [F, F], BF16)
        sbu1 = load("bu1", bu1, [F, 1])
        sWu2 = load("Wu2", Wu2, [F, F], BF16)
        sWp1n = load("Wp1n", Wp1n, [F, F], BF16)
        sWp1g = load("Wp1g", Wp1g, [F, F])
        sbp1 = load("bp1", bp1, [F, 1])
        sWp2 = load("Wp2", Wp2, [F, F], BF16)
        sWo1 = load("Wo1", Wo1, [F, F])
        sbo1 = load("bo1", bo1, [F, 1])
        sWo2 = load("Wo2", Wo2, [F, OUT], BF16)
        stau = load("tau", tau_d, [1, 1])
        if not zero_bias:
            sbe2_4 = load("be2_4", be2_4, [1, 4 * F], BF16)
            sbu2_4 = load("bu2_4", bu2_4, [1, 4 * F], BF16)
            sbp2_4 = load("bp2_4", bp2_4, [1, 4 * F], BF16)
            sbo2 = load("bo2", bo2, [1, OUT], BF16)

        ident = singles.tile([128, 128], F32, tag="ident")
        make_identity(nc, ident[:])
        ident16 = singles.tile([128, 128], BF16, tag="ident16")
        nc.vector.tensor_copy(out=ident16[:], in_=ident[:])
        ones_r = singles.tile([1, PB], BF16, tag="ones_r")
        nc.vector.memset(ones_r[:], 1.0)
        ones_r32 = singles.tile([1, 128], F32, tag="ones_r32")
        nc.vector.memset(ones_r32[:], 1.0)

        # SBUF-resident aggregation tables (written per group, read per block)
        aggS1 = singles.tile([128, NG * G], BF16, tag="aggS1")
        aggS2 = singles.tile([128, NG * G], BF16, tag="aggS2")

        magic = singles.tile([128, 8], I32, tag="magic")
        nc.vector.memset(magic[:], 0x5F3759DF)

        # ---------- conditioning nets (tau -> scale/shift) + weight folds ----------
        cvec = {}
        for k in ("e", "u", "p"):
            C1, c1, C2a, C2b, c2a1, c2b = cond_w[k]
            sC1 = load(f"C{k}1", C1, [1, H])
            sc1 = load(f"c{k}1", c1, [H, 1])
            sC2a = load(f"C{k}2a", C2a, [H, F])
            sC2b = load(f"C{k}2b", C2b, [H, F])
            sc2a1 = load(f"c{k}2a1", c2a1, [F, 1])
            sc2b = load(f"c{k}2b", c2b, [F, 1])

            ph = psS.tile([H, 1], F32, tag="pS")
            nc.tensor.matmul(ph[:], lhsT=sC1[:], rhs=stau[:], start=True, stop=True)
            hs = small.tile([H, 1], F32, tag=f"hs{k}")
            nc.scalar.activation(hs[:], ph[:], AF.Silu, bias=sc1[:], scale=1.0)

            pscale = psS.tile([F, 1], F32, tag="pS")
            nc.tensor.matmul(pscale[:], lhsT=sC2a[:], rhs=hs[:], start=True, stop=True)
            s1p = singles.tile([F, 1], F32, tag=f"s1p{k}")
            nc.vector.tensor_scalar(
                out=s1p[:], in0=pscale[:], scalar1=sc2a1[:], scalar2=None, op0=ALU.add
            )
            pshift = psS.tile([F, 1], F32, tag="pS")
            nc.tensor.matmul(pshift[:], lhsT=sC2b[:], rhs=hs[:], start=True, stop=True)
            shift = singles.tile([F, 1], F32, tag=f"shift{k}")
            nc.vector.tensor_scalar(
                out=shift[:], in0=pshift[:], scalar1=sc2b[:], scalar2=None, op0=ALU.add
            )
            cvec[k] = (s1p, shift)

        s1pe, shifte = cvec["e"]
        s1pu, shiftu = cvec["u"]
        s1pp, shiftp = cvec["p"]

        # fold cond-norm affines into downstream weights
        fWu1a = singles.tile([F, F], BF16, tag="fWu1a")
        nc.vector.tensor_tensor(
            out=fWu1a[:], in0=sWu1a[:], in1=s1pe[:].to_broadcast([F, F]), op=ALU.mult
        )
        pbu1 = psS.tile([F, 1], F32, tag="pS")
        nc.tensor.matmul(pbu1[:], lhsT=sWu1a[:], rhs=shifte[:], start=True, stop=True)
        fbu1 = singles.tile([F, 1], F32, tag="fbu1")
        nc.vector.tensor_scalar(
            out=fbu1[:], in0=pbu1[:], scalar1=sbu1[:], scalar2=None, op0=ALU.add
        )

        fWp1ge = singles.tile([F, F], BF16, tag="fWp1ge")
        nc.vector.tensor_tensor(
            out=fWp1ge[:], in0=sWp1g[:], in1=s1pe[:].to_broadcast([F, F]), op=ALU.mult
        )
        fWp1gu = singles.tile([F, F], BF16, tag="fWp1gu")
        nc.vector.tensor_tensor(
            out=fWp1gu[:], in0=sWp1g[:], in1=s1pu[:].to_broadcast([F, F]), op=ALU.mult
        )
        shifteu = small.tile([F, 1], F32, tag="shifteu")
        nc.vector.tensor_tensor(
            out=shifteu[:], in0=shifte[:], in1=shiftu[:], op=ALU.add
        )
        pbpe = psS.tile([1, F], F32, tag="pS")
        nc.tensor.matmul(pbpe[:], lhsT=shifteu[:], rhs=sWp1g[:], start=True, stop=True)
        bpe_row = singles.tile([1, F], BF16, tag="bpe_row")
        nc.vector.tensor_copy(out=bpe_row[:], in_=pbpe[:])

        fWo1 = singles.tile([F, F], BF16, tag="fWo1")
        nc.vector.tensor_tensor(
            out=fWo1[:], in0=sWo1[:], in1=s1pp[:].to_broadcast([F, F]), op=ALU.mult
        )
        sWo116 = singles.tile([F, F], BF16, tag="sWo116")
        nc.vector.tensor_copy(out=sWo116[:], in_=sWo1[:])
        pbo1 = psS.tile([F, 1], F32, tag="pS")
        nc.tensor.matmul(pbo1[:], lhsT=sWo1[:], rhs=shiftp[:], start=True, stop=True)
        fbo1 = singles.tile([F, 1], F32, tag="fbo1")
        nc.vector.tensor_scalar(
            out=fbo1[:], in0=pbo1[:], scalar1=sbo1[:], scalar2=None, op0=ALU.add
        )

        # second MLP layer: 4 chunk matmuls (+ bias rank-1s only if nonzero)
        def emit_l2(psum4, ysrc, W, bias4):
            for c in range(4):
                nc.tensor.matmul(
                    psum4[:, c, :],
                    lhsT=ysrc[:, c * 128 : (c + 1) * 128],
                    rhs=W[:],
                    start=True,
                    stop=zero_bias,
                )
                if not zero_bias:
                    nc.tensor.matmul(
                        psum4[:, c, :], lhsT=ones_r[:, 0:128],
                        rhs=bias4[:, 0:128], start=False, stop=True,
                    )

        # layer-norm split: per-chunk bn stats into a shared mv tile, one
        # batched rsqrt chain (bitcast seed + 1 NR) for up to 2 LNs, then
        # per-chunk applies (3 on scalar via Identity, 1 on vector).
        magic8 = magic  # [128, 8] int32 0x5f3759df

        def ln_stats(psum4, mv8, half, tag):
            st = small.tile([128, 4, 6], F32, tag=f"st{tag}", name="st")
            for c in range(4):
                nc.vector.bn_stats(out=st[:, c, :], in_=psum4[:, c, :])
            for c in range(4):
                nc.vector.bn_aggr(out=mv8[:, half * 4 + c, :], in_=st[:, c, :])

        def ln_rsqrt(mv8, W, tag):
            vpe = small.tile([128, W], F32, tag=f"vp{tag}", name="vpe")
            nc.vector.tensor_scalar(
                out=vpe[:], in0=mv8[:, 0:W, 1], scalar1=EPS, scalar2=None,
                op0=ALU.add,
            )
            ish = small.tile([128, W], I32, tag=f"is{tag}", name="ish")
            nc.vector.tensor_scalar(
                out=ish[:], in0=vpe[:].bitcast(I32), scalar1=1, scalar2=None,
                op0=ALU.arith_shift_right,
            )
            y0i = small.tile([128, W], I32, tag=f"y0{tag}", name="y0i")
            nc.vector.tensor_tensor(
                out=y0i[:], in0=magic8[:, 0:W], in1=ish[:], op=ALU.subtract
            )
            y0 = y0i[:].bitcast(F32)
            y0sq = small.tile([128, W], F32, tag=f"yq{tag}", name="y0sq")
            nc.vector.tensor_tensor(out=y0sq[:], in0=y0, in1=y0, op=ALU.mult)
            th = small.tile([128, W], F32, tag=f"th{tag}", name="th")
            nc.vector.scalar_tensor_tensor(
                out=th[:], in0=y0sq[:], scalar=-0.5, in1=vpe[:],
                op0=ALU.mult, op1=ALU.mult,
            )
            rstd = small.tile([128, W], F32, tag=f"rs{tag}", name="rstd")
            nc.vector.scalar_tensor_tensor(
                out=rstd[:], in0=th[:], scalar=1.5, in1=y0,
                op0=ALU.add, op1=ALU.mult,
            )
            nmr = small.tile([128, W], F32, tag=f"nm{tag}", name="nmr")
            nc.vector.scalar_tensor_tensor(
                out=nmr[:], in0=mv8[:, 0:W, 0], scalar=-1.0, in1=rstd[:],
                op0=ALU.mult, op1=ALU.mult,
            )
            return rstd, nmr

        def ln_apply(psum4, out4, rstd, nmr, mv8, half):
            for c in range(4):
                w = half * 4 + c
                if c % 2 == 1:
                    nc.vector.tensor_scalar(
                        out=out4[:, c, :],
                        in0=psum4[:, c, :],
                        scalar1=mv8[:, w, 0:1],
                        scalar2=rstd[:, w : w + 1],
                        op0=ALU.subtract,
                        op1=ALU.mult,
                    )
                else:
                    nc.scalar.activation(
                        out4[:, c, :], psum4[:, c, :], AF.Identity,
                        bias=nmr[:, w : w + 1], scale=rstd[:, w : w + 1],
                    )

        # transpose [128, 4, 128] sbuf bf16 -> [128, 512] sbuf bf16 (feat, slot)
        def transp(ln4, outT_t):
            ptr = psT.tile([128, 4, 128], BF16, tag="pT", name="ptr")
            for c in range(4):
                nc.tensor.transpose(ptr[:, c, :], ln4[:, c, :], ident16[:])
            for c in range(4):
                if c % 2 == 0:
                    nc.vector.tensor_copy(
                        out=outT_t[:, c * 128 : (c + 1) * 128], in_=ptr[:, c, :]
                    )
                else:
                    nc.scalar.copy(
                        out=outT_t[:, c * 128 : (c + 1) * 128], in_=ptr[:, c, :]
                    )

        # ---------- edge phase (software-pipelined across groups) ----------
        st_macro = {}   # macro index -> dict of macro tiles
        st_grp = {}     # group index -> dict of per-group tiles

        def macro_load(m):
            g0 = m * M
            gn = min(M, NG - g0)
            ew = gn * EPG
            t = {}
            t["ef"] = macro.tile([EIN, M * EPG], BF16, tag="efM", name="efM")
            nc.sync.dma_start(
                out=t["ef"][:, 0:ew], in_=efT_d[:, g0 * EPG : g0 * EPG + ew]
            )
            t["sf"] = macro.tile([F, M * EPG], BF16, tag="sfM", name="sfM")
            nc.sync.dma_start(
                out=t["sf"][:, 0:ew], in_=sfT_d[:, g0 * EPG : g0 * EPG + ew]
            )
            t["rf"] = macro.tile([F, M * EPG], BF16, tag="rfM", name="rfM")
            nc.sync.dma_start(
                out=t["rf"][:, 0:ew], in_=rfT_d[:, g0 * EPG : g0 * EPG + ew]
            )
            t["oh"] = macro.tile([128, M * 4 * G], BF16, tag="ohM", name="ohM")
            nc.sync.dma_start(
                out=t["oh"][:, 0 : gn * 4 * G],
                in_=oh_d[:, g0 * 4 * G : (g0 + gn) * 4 * G],
            )
            st_macro[m] = t

        def s1_embed(g):
            mt = st_macro[g // M]
            esl = slice((g % M) * EPG, (g % M + 1) * EPG)
            d = st_grp.setdefault(g, {})
            d["esl"] = esl
            pz1 = psB.tile([128, EPG], F32, tag="pB")
            nc.tensor.matmul(
                pz1[:], lhsT=sWe1[:], rhs=mt["ef"][:, esl], start=True, stop=True
            )
            y1 = work.tile([128, EPG], BF16, tag="y1")
            nc.scalar.activation(y1[:], pz1[:], AF.Silu, bias=sbe1[:], scale=1.0)
            pz2 = psA.tile([128, 4, 128], F32, tag="pA")
            emit_l2(pz2, y1, sWe2, None if zero_bias else sbe2_4)
            d["pz2"] = pz2

        def s2_lne(g):
            d = st_grp[g]
            mv8 = small.tile([128, 4, 2], F32, tag="mvE", name="mv8")
            ln_stats(d["pz2"], mv8, 0, "e")
            rstd, nmr = ln_rsqrt(mv8, 4, "e")
            ln1 = work3.tile([128, 4, 128], BF16, tag="ln1", name="ln1")
            ln_apply(d["pz2"], ln1, rstd, nmr, mv8, 0)
            d["ln1"] = ln1
            del d["pz2"]
            ln1T = work.tile([128, EPG], BF16, tag="ln1T", name="ln1T")
            transp(ln1, ln1T)
            d["ln1T"] = ln1T

        def s4_lnu(g):
            d = st_grp[g]
            mv8 = small.tile([128, 4, 2], F32, tag="mvU", name="mv8")
            ln_stats(d["pu2"], mv8, 0, "u")
            rstd, nmr = ln_rsqrt(mv8, 4, "u")
            ln2 = work.tile([128, 4, 128], BF16, tag="ln2", name="ln2")
            ln_apply(d["pu2"], ln2, rstd, nmr, mv8, 0)
            d["ln2"] = ln2
            del d["pu2"]

        def ln_driver(i):
            if 1 <= i and i - 1 < NG:
                s2_lne(i - 1)
            if 2 <= i and i - 2 < NG:
                s4_lnu(i - 2)

        def s3_update(g):
            d = st_grp[g]
            mt = st_macro[g // M]
            esl = d["esl"]
            pu1 = psB.tile([128, EPG], F32, tag="pB")
            nc.tensor.matmul(
                pu1[:], lhsT=fWu1a[:], rhs=d["ln1T"][:], start=True, stop=False
            )
            nc.tensor.matmul(
                pu1[:], lhsT=sWu1b[:], rhs=mt["sf"][:, esl], start=False, stop=False
            )
            nc.tensor.matmul(
                pu1[:], lhsT=sWu1c[:], rhs=mt["rf"][:, esl], start=False, stop=True
            )
            del d["ln1T"]
            yu = work.tile([128, EPG], BF16, tag="yu")
            nc.scalar.activation(yu[:], pu1[:], AF.Silu, bias=fbu1[:], scale=1.0)
            pu2 = psA.tile([128, 4, 128], F32, tag="pA")
            emit_l2(pu2, yu, sWu2, None if zero_bias else sbu2_4)
            d["pu2"] = pu2

        def s5_agg(g):
            d = st_grp[g]
            mt = st_macro[g // M]
            gm = g % M
            Sps = psS.tile([128, 2 * G], F32, tag="pS")
            for c in range(4):
                nc.tensor.matmul(
                    Sps[:, 0:G],
                    lhsT=d["ln1"][:, c, :],
                    rhs=mt["oh"][:, (gm * 4 + c) * G : (gm * 4 + c + 1) * G],
                    start=(c == 0),
                    stop=(c == 3),
                )
            for c in range(4):
                nc.tensor.matmul(
                    Sps[:, G : 2 * G],
                    lhsT=d["ln2"][:, c, :],
                    rhs=mt["oh"][:, (gm * 4 + c) * G : (gm * 4 + c + 1) * G],
                    start=(c == 0),
                    stop=(c == 3),
                )
            nc.vector.tensor_copy(
                out=aggS1[:, g * G : (g + 1) * G], in_=Sps[:, 0:G]
            )
            nc.scalar.copy(
                out=aggS2[:, g * G : (g + 1) * G], in_=Sps[:, G : 2 * G]
            )
            del st_grp[g]

        NMAC = (NG + M - 1) // M
        macro_load(0)
        for i in range(NG + 3):
            ln_driver(i)
            if i >= 3:
                s5_agg(i - 3)
            if i < NG:
                if i % M == 0 and (i // M) + 1 < NMAC:
                    macro_load(i // M + 1)
                s1_embed(i)
            if i >= 1 and i - 1 < NG:
                s3_update(i - 1)

        # ---------- pnode phase (software-pipelined across blocks) ----------
        OBW = 4  # blocks per staging window
        st_blk = {}

        def win_load(wi):
            w = {}
            w["inv"] = macroP.tile([1, OBW * PB], F32, tag="invW", name="invW")
            nc.sync.dma_start(
                out=w["inv"][:], in_=inv_d[:, wi * OBW * PB : (wi + 1) * OBW * PB]
            )
            w["m01"] = macroP.tile([1, OBW * PB], BF16, tag="m01W", name="m01W")
            nc.sync.dma_start(
                out=w["m01"][:], in_=m01_d[:, wi * OBW * PB : (wi + 1) * OBW * PB]
            )
            st_blk["w%d" % wi] = w

        def p1_front(j):
            d = st_blk.setdefault(j, {})
            sl = slice(j * PB, (j + 1) * PB)
            d["sl"] = sl
            if j % OBW == 0:
                if j == 0:
                    win_load(0)
                if j // OBW + 1 < NPB // OBW:
                    win_load(j // OBW + 1)
            w = st_blk["w%d" % (j // OBW)]
            wsl = slice((j % OBW) * PB, (j % OBW + 1) * PB)

            pinv = psT.tile([128, PB], F32, tag="pT")
            nc.tensor.matmul(
                pinv[:], lhsT=ones_r32[:], rhs=w["inv"][:, wsl],
                start=True, stop=True,
            )
            invb = work.tile([128, PB], F32, tag="invb")
            nc.scalar.copy(out=invb[:], in_=pinv[:])

            pA = psB.tile([128, PB], F32, tag="pB")
            nc.tensor.matmul(
                pA[:], lhsT=fWp1ge[:], rhs=aggS1[:, sl], start=True, stop=False
            )
            nc.tensor.matmul(
                pA[:], lhsT=fWp1gu[:], rhs=aggS2[:, sl], start=False, stop=True
            )
            tA = work.tile([128, PB], BF16, tag="tA")
            nc.vector.tensor_tensor(out=tA[:], in0=pA[:], in1=invb[:], op=ALU.mult)

            pzp = psB.tile([128, PB], F32, tag="pB")
            nc.tensor.matmul(
                pzp[:], lhsT=sWp1n[:], rhs=pnT[:, sl], start=True, stop=False
            )
            nc.tensor.matmul(
                pzp[:], lhsT=ident16[:], rhs=tA[:], start=False, stop=False
            )
            nc.tensor.matmul(
                pzp[:], lhsT=bpe_row[:], rhs=w["m01"][:, wsl],
                start=False, stop=True,
            )
            yp = work.tile([128, PB], BF16, tag="yp")
            nc.scalar.activation(yp[:], pzp[:], AF.Silu, bias=sbp1[:], scale=1.0)

            pp2 = psA.tile([128, 4, 128], F32, tag="pA")
            emit_l2(pp2, yp, sWp2, None if zero_bias else sbp2_4)
            d["pp2"] = pp2

        def p2_ln(j):
            d = st_blk[j]
            mv8 = small.tile([128, 8, 2], F32, tag="mv8", name="mv8")
            ln_stats(d["pp2"], mv8, 0, "p")
            rstd, nmr = ln_rsqrt(mv8, 4, "p")
            lnp = work3.tile([128, 4, 128], BF16, tag="ln1", name="lnp")
            ln_apply(d["pp2"], lnp, rstd, nmr, mv8, 0)
            del d["pp2"]
            lnpT = work.tile([128, PB], BF16, tag="ln1T", name="lnpT")
            transp(lnp, lnpT)
            d["lnpT"] = lnpT

        def p3_out(j):
            d = st_blk[j]
            sl = d["sl"]
            lnpT = d["lnpT"]
            pzo = psB.tile([128, PB], F32, tag="pB")
            nc.tensor.matmul(
                pzo[:], lhsT=fWo1[:], rhs=lnpT[:], start=True, stop=False
            )
            nc.tensor.matmul(
                pzo[:], lhsT=sWo116[:], rhs=pnT[:, sl], start=False, stop=True
            )
            yo = work.tile([128, PB], BF16, tag="yo")
            nc.scalar.activation(yo[:], pzo[:], AF.Silu, bias=fbo1[:], scale=1.0)

            po = psS.tile([OUT, PB], F32, tag="pS")
            nc.tensor.matmul(
                po[:], lhsT=sWo2[:], rhs=yo[:], start=True, stop=zero_bias
            )
            if not zero_bias:
                nc.tensor.matmul(
                    po[:], lhsT=sbo2[:], rhs=ones_r[:], start=False, stop=True
                )
            if j % OBW == 0:
                d2 = st_blk.setdefault("ob%d" % (j // OBW), {})
                d2["ob"] = macroP.tile([OUT, OBW * PB], F32, tag="ob", name="ob")
            ob = st_blk["ob%d" % (j // OBW)]["ob"]
            nc.vector.tensor_copy(
                out=ob[:, (j % OBW) * PB : (j % OBW + 1) * PB], in_=po[:]
            )
            if j % OBW == OBW - 1:
                nc.sync.dma_start(
                    out=outT[:, (j - OBW + 1) * PB : (j + 1) * PB], in_=ob[:]
                )
            del st_blk[j]

        for j in range(NPB + 2):
            if j >= 1 and j - 1 < NPB:
                p2_ln(j - 1)
            if j >= 2:
                p3_out(j - 2)
            if j < NPB:
                p1_front(j)

    nc.compile()
    return nc


def _prep_core(ef_b, snd_b, rcv_b, rn_b, pn_b, tau_b, q):
    import ml_dtypes

    lo = q * QP
    mask = (rcv_b >= lo) & (rcv_b < lo + QP)
    ed = np.nonzero(mask)[0]
    loc = (rcv_b[ed] - lo).astype(np.int64)
    order = np.argsort(loc, kind="stable")
    ed, loc = ed[order], loc[order]
    grp = loc // G
    cnts = np.bincount(grp, minlength=NG)
    assert cnts.max() <= EPG, f"group overflow: {cnts.max()} > {EPG}"
    gstart = np.concatenate([[0], np.cumsum(cnts)[:-1]])
    slot = grp * EPG + (np.arange(len(ed)) - gstart[grp])

    efp = np.zeros((NEP, EIN), np.float32)
    efp[slot] = ef_b[ed]
    # host pre-gather of sender/receiver features per edge slot
    sfp = np.zeros((NEP, F), np.float32)
    sfp[slot] = rn_b[snd_b[ed]]
    rfp = np.zeros((NEP, F), np.float32)
    rfp[slot] = pn_b[rcv_b[ed]]
    rrel = np.full(NEP, -1.0, np.float32)
    rrel[slot] = (loc - grp * G).astype(np.float32)
    # precomputed one-hot [slot -> segment] per group, laid out
    # [128 partitions, NG, 4 chunks, G] with slot = chunk*128 + partition
    ohm = (
        rrel.reshape(NG, 4, 128, 1) == np.arange(G, dtype=np.float32)
    ).astype(np.float32)
    ohm = ohm.transpose(2, 0, 1, 3).reshape(128, NG * 4 * G)

    cnt = np.bincount(loc, minlength=QP).astype(np.float32)
    inv = (1.0 / np.maximum(cnt, 1.0)).astype(np.float32)
    m01 = np.minimum(cnt, 1.0)

    pn_q = pn_b[lo : lo + QP]
    return {
        "efT": np.ascontiguousarray(efp.T.astype(ml_dtypes.bfloat16)),
        "sfT": np.ascontiguousarray(sfp.T.astype(ml_dtypes.bfloat16)),
        "rfT": np.ascontiguousarray(rfp.T.astype(ml_dtypes.bfloat16)),
        "ohM": np.ascontiguousarray(ohm).astype(ml_dtypes.bfloat16),
        "pnT": np.ascontiguousarray(pn_q.T).astype(ml_dtypes.bfloat16),
        "invQ": inv.reshape(1, QP),
        "m01Q": m01.reshape(1, QP).astype(ml_dtypes.bfloat16),
        "tau": tau_b.reshape(1, 1).astype(np.float32),
    }


def _prep_weights(i):
    w = {
        "We1": i["We1"], "be1": i["be1"].reshape(F, 1), "We2": i["We2"],
        "be2_4": np.tile(i["be2"].reshape(1, F), (1, 4)),
        "Wu1a": i["Wu1"][0:F], "Wu1b": i["Wu1"][F : 2 * F],
        "Wu1c": i["Wu1"][2 * F : 3 * F],
        "bu1": i["bu1"].reshape(F, 1), "Wu2": i["Wu2"],
        "bu2_4": np.tile(i["bu2"].reshape(1, F), (1, 4)),
        "Wp1n": i["Wp1"][0:F], "Wp1g": i["Wp1"][F : 2 * F],
        "bp1": i["bp1"].reshape(F, 1), "Wp2": i["Wp2"],
        "bp2_4": np.tile(i["bp2"].reshape(1, F), (1, 4)),
        "Wo1": i["Wo1"], "bo1": i["bo1"].reshape(F, 1), "Wo2": i["Wo2"],
        "bo2": i["bo2"].reshape(1, OUT),
    }
    for k in ("e", "u", "p"):
        C1, c1 = i[f"C{k}1"], i[f"c{k}1"]
        C2, c2 = i[f"C{k}2"], i[f"c{k}2"]
        w[f"C{k}1"] = C1.reshape(1, H)
        w[f"c{k}1"] = c1.reshape(H, 1)
        w[f"C{k}2a"] = np.ascontiguousarray(C2[:, 0:F])
        w[f"C{k}2b"] = np.ascontiguousarray(C2[:, F : 2 * F])
        w[f"c{k}2a1"] = (c2[0:F] + 1.0).reshape(F, 1)
        w[f"c{k}2b"] = c2[F : 2 * F].reshape(F, 1)
    import ml_dtypes

    bf16_keys = {"We1", "We2", "Wu1b", "Wu1c", "Wu2", "Wp1n", "Wp2", "Wo2",
                 "be2_4", "bu2_4", "bp2_4", "bo2"}
    return {
        k: np.ascontiguousarray(
            v, dtype=ml_dtypes.bfloat16 if k in bf16_keys else np.float32
        )
        for k, v in w.items()
    }


_NC_CACHE = {}


def _zero_bias(i):
    return all(
        float(np.abs(np.asarray(i[k])).max()) == 0.0
        for k in ("be2", "bu2", "bp2", "bo2")
    )


def build_in_maps(inputs):
    i = {k: np.asarray(v) for k, v in inputs.items()}
    w = _prep_weights(i)
    in_maps = []
    for core in range(8):
        b, q = core // NQ, core % NQ
        m = dict(w)
        m.update(
            _prep_core(
                i["edge_features"][b], i["senders"][b], i["receivers"][b],
                i["rnode_features"][b], i["pnode_features"][b], i["tau"][b], q
            )
        )
        in_maps.append(m)
    return in_maps


def get_nc(zero_bias=True):
    key = ("nc", bool(zero_bias))
    if key not in _NC_CACHE:
        _NC_CACHE[key] = _build_nc(bool(zero_bias))
    return _NC_CACHE[key]


def assemble(results):
    out = np.zeros((B, NPTOT, OUT), np.float32)
    for core in range(8):
        b, q = core // NQ, core % NQ
        out[b, q * QP : (q + 1) * QP, :] = results[core]["outT"].T
    return out


def kernel(**inputs):
    from concourse.bass_utils import run_bass_kernel_spmd

    i = {k: np.asarray(v) for k, v in inputs.items()}
    nc = get_nc(_zero_bias(i))
    in_maps = build_in_maps(i)
    res = run_bass_kernel_spmd(nc, in_maps, list(range(8)))
    return assemble(res.results)


if __name__ == "__main__":
    import reference

    inputs = reference.setup_inputs()
    out = kernel(**{k: np.asarray(v) for k, v in inputs.items()})
    print("out", out.shape, out.dtype)


# revision 37
# speedup vs baseline: 1.6813x; 1.0302x over previous
"""Trainium2 Bass kernel for the GNN decoder (message passing, cond-layernorm).

Sharding: 8 cores = (batch b in {0,1}) x (pnode quarter q in {0..3}).
Each core owns pnode rows [q*16384, (q+1)*16384) of its batch and every edge
whose receiver lands in that range.  Edges are receiver-sorted on the host and
packed into NG groups of G=104 consecutive segments with a fixed budget of
EPG=512 edge slots per group (padded; pad slots have one-hot row == 0 so they
contribute nothing).

Host-side prep pre-gathers sender/receiver features per edge slot (sfT/rfT,
bf16, streamed from DRAM) so the device loop needs no gpsimd gathers.  The
edge loop is software-pipelined across groups (stages S1..S5 emitted with
skews 0/1/1/2/3) so each engine's in-order queue always has ready work; the
pnode loop is pipelined the same way.  Segment sums accumulate via one-hot
matmuls into SBUF-resident aggregate tables consumed directly by the pnode
phase.  Cond-norm affines are folded into downstream weights on device once
per launch.  The scalar engine only ever runs {silu, identity, copy} (one act
table set); layernorm rsqrt runs on the vector engine via a quake-style
bitcast seed plus one Newton step.  MLP bias rank-1 matmuls are emitted only
if the corresponding host-side bias vectors are nonzero (they are
structurally zero in this model).
"""

import numpy as np

import concourse.bass as bass
import concourse.tile as tile
from concourse import bacc
from concourse import mybir
from concourse.masks import make_identity

F32 = mybir.dt.float32
BF16 = mybir.dt.bfloat16
I32 = mybir.dt.int32

B, NR, NPTOT, E, F, EIN, H, OUT = 2, 16384, 65536, 262144, 128, 4, 16, 4
EPS = 1e-6
NQ = 4                  # pnode quarters per batch
QP = NPTOT // NQ        # pnodes per core (16384)
G = 104                 # segments per group
EPG = 512               # edge slots per group
NG = (QP + G - 1) // G  # groups per core (158)
NEP = NG * EPG          # padded edge slots per core
PB = 512                # pnode block width
NPB = QP // PB          # pnode blocks per core (32)
M = 2                   # groups per macro DMA batch

AF = mybir.ActivationFunctionType
ALU = mybir.AluOpType


def _build_nc(zero_bias):
    nc = bacc.Bacc("TRN2", target_bir_lowering=False, debug=False)

    def inp(name, shape, dtype=F32):
        return nc.dram_tensor(name, shape, dtype, kind="ExternalInput")

    efT_d = inp("efT", [EIN, NEP], BF16)
    sfT_d = inp("sfT", [F, NEP], BF16)
    rfT_d = inp("rfT", [F, NEP], BF16)
    oh_d = inp("ohM", [128, NG * 4 * G], BF16)
    pnT_d = inp("pnT", [F, QP], BF16)
    tau_d = inp("tau", [1, 1])
    inv_d = inp("invQ", [1, QP])
    m01_d = inp("m01Q", [1, QP], BF16)

    We1 = inp("We1", [EIN, F], BF16)
    be1 = inp("be1", [F, 1])
    We2 = inp("We2", [F, F], BF16)
    be2_4 = inp("be2_4", [1, 4 * F], BF16)
    Wu1a = inp("Wu1a", [F, F])
    Wu1b = inp("Wu1b", [F, F], BF16)
    Wu1c = inp("Wu1c", [F, F], BF16)
    bu1 = inp("bu1", [F, 1])
    Wu2 = inp("Wu2", [F, F], BF16)
    bu2_4 = inp("bu2_4", [1, 4 * F], BF16)
    Wp1n = inp("Wp1n", [F, F], BF16)
    Wp1g = inp("Wp1g", [F, F])
    bp1 = inp("bp1", [F, 1])
    Wp2 = inp("Wp2", [F, F], BF16)
    bp2_4 = inp("bp2_4", [1, 4 * F], BF16)
    Wo1 = inp("Wo1", [F, F])
    bo1 = inp("bo1", [F, 1])
    Wo2 = inp("Wo2", [F, OUT], BF16)
    bo2 = inp("bo2", [1, OUT], BF16)
    # conditioning nets: e(dge embed), u(pdate), p(node).  r is dead code.
    cond_w = {}
    for k in ("e", "u", "p"):
        cond_w[k] = (
            inp(f"C{k}1", [1, H]),
            inp(f"c{k}1", [H, 1]),
            inp(f"C{k}2a", [H, F]),     # scale half of C2
            inp(f"C{k}2b", [H, F]),     # shift half of C2
            inp(f"c{k}2a1", [F, 1]),    # c2[:F] + 1.0
            inp(f"c{k}2b", [F, 1]),     # c2[F:]
        )

    outT = nc.dram_tensor("outT", [OUT, QP], F32, kind="ExternalOutput")

    from contextlib import ExitStack

    with tile.TileContext(nc) as tc, ExitStack() as ctx:
        singles = ctx.enter_context(tc.tile_pool(name="singles", bufs=1))
        macro = ctx.enter_context(tc.tile_pool(name="macro", bufs=4))
        macroP = ctx.enter_context(tc.tile_pool(name="macroP", bufs=2))
        work = ctx.enter_context(tc.tile_pool(name="work", bufs=2))
        work3 = ctx.enter_context(tc.tile_pool(name="work3", bufs=3))
        small = ctx.enter_context(tc.tile_pool(name="small", bufs=4))
        psB = ctx.enter_context(tc.tile_pool(name="psB", bufs=2, space="PSUM"))
        psA = ctx.enter_context(tc.tile_pool(name="psA", bufs=2, space="PSUM"))
        psT = ctx.enter_context(tc.tile_pool(name="psT", bufs=2, space="PSUM"))
        psS = ctx.enter_context(tc.tile_pool(name="psS", bufs=2, space="PSUM"))

        # ---------- constants & resident tables ----------
        def load(name, dram, shape, dtype=F32):
            t = singles.tile(shape, dtype, tag=name)
            nc.sync.dma_start(out=t[:], in_=dram[:])
            return t

        pnT = load("pnT", pnT_d, [F, QP], BF16)
        sWe1 = load("We1", We1, [EIN, F], BF16)
        sbe1 = load("be1", be1, [F, 1])
        sWe2 = load("We2", We2, [F, F], BF16)
        sWu1a = load("Wu1a", Wu1a, [F, F])
        sWu1b = load("Wu1b", Wu1b, [F, F], BF16)
        sWu1c = load("Wu1c", Wu1c, [F, F], BF16)
        sbu1 = load("bu1", bu1, [F, 1])
        sWu2 = load("Wu2", Wu2, [F, F], BF16)
        sWp1n = load("Wp1n", Wp1n, [F, F], BF16)
        sWp1g = load("Wp1g", Wp1g, [F, F])
        sbp1 = load("bp1", bp1, [F, 1])
        sWp2 = load("Wp2", Wp2, [F, F], BF16)
        sWo1 = load("Wo1", Wo1, [F, F])
        sbo1 = load("bo1", bo1, [F, 1])
        sWo2 = load("Wo2", Wo2, [F, OUT], BF16)
        stau = load("tau", tau_d, [1, 1])
        if not zero_bias:
            sbe2_4 = load("be2_4", be2_4, [1, 4 * F], BF16)
            sbu2_4 = load("bu2_4", bu2_4, [1, 4 * F], BF16)
            sbp2_4 = load("bp2_4", bp2_4, [1, 4 * F], BF16)
            sbo2 = load("bo2", bo2, [1, OUT], BF16)

        ident = singles.tile([128, 128], F32, tag="ident")
        make_identity(nc, ident[:])
        ident16 = singles.tile([128, 128], BF16, tag="ident16")
        nc.vector.tensor_copy(out=ident16[:], in_=ident[:])
        ones_r = singles.tile([1, PB], BF16, tag="ones_r")
        nc.vector.memset(ones_r[:], 1.0)
        ones_r32 = singles.tile([1, 128], F32, tag="ones_r32")
        nc.vector.memset(ones_r32[:], 1.0)

        # SBUF-resident aggregation tables (written per group, read per block)
        aggS1 = singles.tile([128, NG * G], BF16, tag="aggS1")
        aggS2 = singles.tile([128, NG * G], BF16, tag="aggS2")

        magic = singles.tile([128, 8], I32, tag="magic")
        nc.vector.memset(magic[:], 0x5F3759DF)

        # ---------- conditioning nets (tau -> scale/shift) + weight folds ----------
        cvec = {}
        for k in ("e", "u", "p"):
            C1, c1, C2a, C2b, c2a1, c2b = cond_w[k]
            sC1 = load(f"C{k}1", C1, [1, H])
            sc1 = load(f"c{k}1", c1, [H, 1])
            sC2a = load(f"C{k}2a", C2a, [H, F])
            sC2b = load(f"C{k}2b", C2b, [H, F])
            sc2a1 = load(f"c{k}2a1", c2a1, [F, 1])
            sc2b = load(f"c{k}2b", c2b, [F, 1])

            ph = psS.tile([H, 1], F32, tag="pS")
            nc.tensor.matmul(ph[:], lhsT=sC1[:], rhs=stau[:], start=True, stop=True)
            hs = small.tile([H, 1], F32, tag=f"hs{k}")
            nc.scalar.activation(hs[:], ph[:], AF.Silu, bias=sc1[:], scale=1.0)

            pscale = psS.tile([F, 1], F32, tag="pS")
            nc.tensor.matmul(pscale[:], lhsT=sC2a[:], rhs=hs[:], start=True, stop=True)
            s1p = singles.tile([F, 1], F32, tag=f"s1p{k}")
            nc.vector.tensor_scalar(
                out=s1p[:], in0=pscale[:], scalar1=sc2a1[:], scalar2=None, op0=ALU.add
            )
            pshift = psS.tile([F, 1], F32, tag="pS")
            nc.tensor.matmul(pshift[:], lhsT=sC2b[:], rhs=hs[:], start=True, stop=True)
            shift = singles.tile([F, 1], F32, tag=f"shift{k}")
            nc.vector.tensor_scalar(
                out=shift[:], in0=pshift[:], scalar1=sc2b[:], scalar2=None, op0=ALU.add
            )
            cvec[k] = (s1p, shift)

        s1pe, shifte = cvec["e"]
        s1pu, shiftu = cvec["u"]
        s1pp, shiftp = cvec["p"]

        # fold cond-norm affines into downstream weights
        fWu1a = singles.tile([F, F], BF16, tag="fWu1a")
        nc.vector.tensor_tensor(
            out=fWu1a[:], in0=sWu1a[:], in1=s1pe[:].to_broadcast([F, F]), op=ALU.mult
        )
        pbu1 = psS.tile([F, 1], F32, tag="pS")
        nc.tensor.matmul(pbu1[:], lhsT=sWu1a[:], rhs=shifte[:], start=True, stop=True)
        fbu1 = singles.tile([F, 1], F32, tag="fbu1")
        nc.vector.tensor_scalar(
            out=fbu1[:], in0=pbu1[:], scalar1=sbu1[:], scalar2=None, op0=ALU.add
        )

        fWp1ge = singles.tile([F, F], BF16, tag="fWp1ge")
        nc.vector.tensor_tensor(
            out=fWp1ge[:], in0=sWp1g[:], in1=s1pe[:].to_broadcast([F, F]), op=ALU.mult
        )
        fWp1gu = singles.tile([F, F], BF16, tag="fWp1gu")
        nc.vector.tensor_tensor(
            out=fWp1gu[:], in0=sWp1g[:], in1=s1pu[:].to_broadcast([F, F]), op=ALU.mult
        )
        shifteu = small.tile([F, 1], F32, tag="shifteu")
        nc.vector.tensor_tensor(
            out=shifteu[:], in0=shifte[:], in1=shiftu[:], op=ALU.add
        )
        pbpe = psS.tile([1, F], F32, tag="pS")
        nc.tensor.matmul(pbpe[:], lhsT=shifteu[:], rhs=sWp1g[:], start=True, stop=True)
        bpe_row = singles.tile([1, F], BF16, tag="bpe_row")
        nc.vector.tensor_copy(out=bpe_row[:], in_=pbpe[:])

        fWo1 = singles.tile([F, F], BF16, tag="fWo1")
        nc.vector.tensor_tensor(
            out=fWo1[:], in0=sWo1[:], in1=s1pp[:].to_broadcast([F, F]), op=ALU.mult
        )
        sWo116 = singles.tile([F, F], BF16, tag="sWo116")
        nc.vector.tensor_copy(out=sWo116[:], in_=sWo1[:])
        pbo1 = psS.tile([F, 1], F32, tag="pS")
        nc.tensor.matmul(pbo1[:], lhsT=sWo1[:], rhs=shiftp[:], start=True, stop=True)
        fbo1 = singles.tile([F, 1], F32, tag="fbo1")
        nc.vector.tensor_scalar(
            out=fbo1[:], in0=pbo1[:], scalar1=sbo1[:], scalar2=None, op0=ALU.add
        )

        # second MLP layer: 4 chunk matmuls (+ bias rank-1s only if nonzero)
        def emit_l2(psum4, ysrc, W, bias4):
            for c in range(4):
                nc.tensor.matmul(
                    psum4[:, c, :],
                    lhsT=ysrc[:, c * 128 : (c + 1) * 128],
                    rhs=W[:],
                    start=True,
                    stop=zero_bias,
                )
                if not zero_bias:
                    nc.tensor.matmul(
                        psum4[:, c, :], lhsT=ones_r[:, 0:128],
                        rhs=bias4[:, 0:128], start=False, stop=True,
                    )

        # layer-norm split: per-chunk bn stats into a shared mv tile, one
        # batched rsqrt chain (bitcast seed + 1 NR) for up to 2 LNs, then
        # per-chunk applies (3 on scalar via Identity, 1 on vector).
        magic8 = magic  # [128, 8] int32 0x5f3759df

        def ln_stats(psum4, mv8, half, tag):
            st = small.tile([128, 4, 6], F32, tag=f"st{tag}", name="st")
            for c in range(4):
                nc.vector.bn_stats(out=st[:, c, :], in_=psum4[:, c, :])
            for c in range(4):
                nc.vector.bn_aggr(out=mv8[:, half * 4 + c, :], in_=st[:, c, :])

        def ln_rsqrt(mv8, W, tag):
            vpe = small.tile([128, W], F32, tag=f"vp{tag}", name="vpe")
            nc.vector.tensor_scalar(
                out=vpe[:], in0=mv8[:, 0:W, 1], scalar1=EPS, scalar2=None,
                op0=ALU.add,
            )
            ish = small.tile([128, W], I32, tag=f"is{tag}", name="ish")
            nc.vector.tensor_scalar(
                out=ish[:], in0=vpe[:].bitcast(I32), scalar1=1, scalar2=None,
                op0=ALU.arith_shift_right,
            )
            y0i = small.tile([128, W], I32, tag=f"y0{tag}", name="y0i")
            nc.vector.tensor_tensor(
                out=y0i[:], in0=magic8[:, 0:W], in1=ish[:], op=ALU.subtract
            )
            y0 = y0i[:].bitcast(F32)
            y0sq = small.tile([128, W], F32, tag=f"yq{tag}", name="y0sq")
            nc.vector.tensor_tensor(out=y0sq[:], in0=y0, in1=y0, op=ALU.mult)
            th = small.tile([128, W], F32, tag=f"th{tag}", name="th")
            nc.vector.scalar_tensor_tensor(
                out=th[:], in0=y0sq[:], scalar=-0.5, in1=vpe[:],
                op0=ALU.mult, op1=ALU.mult,
            )
            rstd = small.tile([128, W], F32, tag=f"rs{tag}", name="rstd")
            nc.vector.scalar_tensor_tensor(
                out=rstd[:], in0=th[:], scalar=1.5, in1=y0,
                op0=ALU.add, op1=ALU.mult,
            )
            nmr = small.tile([128, W], F32, tag=f"nm{tag}", name="nmr")
            nc.vector.scalar_tensor_tensor(
                out=nmr[:], in0=mv8[:, 0:W, 0], scalar=-1.0, in1=rstd[:],
                op0=ALU.mult, op1=ALU.mult,
            )
            return rstd, nmr

        def ln_apply(psum4, out4, rstd, nmr, mv8, half):
            for c in range(4):
                w = half * 4 + c
                if c % 2 == 1:
                    nc.vector.tensor_scalar(
                        out=out4[:, c, :],
                        in0=psum4[:, c, :],
                        scalar1=mv8[:, w, 0:1],
                        scalar2=rstd[:, w : w + 1],
                        op0=ALU.subtract,
                        op1=ALU.mult,
                    )
                else:
                    nc.scalar.activation(
                        out4[:, c, :], psum4[:, c, :], AF.Identity,
                        bias=nmr[:, w : w + 1], scale=rstd[:, w : w + 1],
                    )

        # transpose [128, 4, 128] sbuf bf16 -> [128, 512] sbuf bf16 (feat, slot)
        def transp(ln4, outT_t):
            ptr = psT.tile([128, 4, 128], BF16, tag="pT", name="ptr")
            for c in range(4):
                nc.tensor.transpose(ptr[:, c, :], ln4[:, c, :], ident16[:])
            for c in range(4):
                if c % 2 == 0:
                    nc.vector.tensor_copy(
                        out=outT_t[:, c * 128 : (c + 1) * 128], in_=ptr[:, c, :]
                    )
                else:
                    nc.scalar.copy(
                        out=outT_t[:, c * 128 : (c + 1) * 128], in_=ptr[:, c, :]
                    )

        # ---------- edge phase (software-pipelined across groups) ----------
        st_macro = {}   # macro index -> dict of macro tiles
        st_grp = {}     # group index -> dict of per-group tiles

        def macro_load(m):
            g0 = m * M
            gn = min(M, NG - g0)
            ew = gn * EPG
            t = {}
            t["ef"] = macro.tile([EIN, M * EPG], BF16, tag="efM", name="efM")
            nc.sync.dma_start(
                out=t["ef"][:, 0:ew], in_=efT_d[:, g0 * EPG : g0 * EPG + ew]
            )
            t["sf"] = macro.tile([F, M * EPG], BF16, tag="sfM", name="sfM")
            nc.sync.dma_start(
                out=t["sf"][:, 0:ew], in_=sfT_d[:, g0 * EPG : g0 * EPG + ew]
            )
            t["rf"] = macro.tile([F, M * EPG], BF16, tag="rfM", name="rfM")
            nc.sync.dma_start(
                out=t["rf"][:, 0:ew], in_=rfT_d[:, g0 * EPG : g0 * EPG + ew]
            )
            t["oh"] = macro.tile([128, M * 4 * G], BF16, tag="ohM", name="ohM")
            nc.sync.dma_start(
                out=t["oh"][:, 0 : gn * 4 * G],
                in_=oh_d[:, g0 * 4 * G : (g0 + gn) * 4 * G],
            )
            st_macro[m] = t

        def s1_embed(g):
            mt = st_macro[g // M]
            esl = slice((g % M) * EPG, (g % M + 1) * EPG)
            d = st_grp.setdefault(g, {})
            d["esl"] = esl
            pz1 = psB.tile([128, EPG], F32, tag="pB")
            nc.tensor.matmul(
                pz1[:], lhsT=sWe1[:], rhs=mt["ef"][:, esl], start=True, stop=True
            )
            y1 = work.tile([128, EPG], BF16, tag="y1")
            nc.scalar.activation(y1[:], pz1[:], AF.Silu, bias=sbe1[:], scale=1.0)
            pz2 = psA.tile([128, 4, 128], F32, tag="pA")
            emit_l2(pz2, y1, sWe2, None if zero_bias else sbe2_4)
            d["pz2"] = pz2

        def s2_lne(g):
            d = st_grp[g]
            mv8 = small.tile([128, 4, 2], F32, tag="mvE", name="mv8")
            ln_stats(d["pz2"], mv8, 0, "e")
            rstd, nmr = ln_rsqrt(mv8, 4, "e")
            ln1 = work3.tile([128, 4, 128], BF16, tag="ln1", name="ln1")
            ln_apply(d["pz2"], ln1, rstd, nmr, mv8, 0)
            d["ln1"] = ln1
            del d["pz2"]
            ln1T = work.tile([128, EPG], BF16, tag="ln1T", name="ln1T")
            transp(ln1, ln1T)
            d["ln1T"] = ln1T

        def s4_lnu(g):
            d = st_grp[g]
            mv8 = small.tile([128, 4, 2], F32, tag="mvU", name="mv8")
            ln_stats(d["pu2"], mv8, 0, "u")
            rstd, nmr = ln_rsqrt(mv8, 4, "u")
            ln2 = work.tile([128, 4, 128], BF16, tag="ln2", name="ln2")
            ln_apply(d["pu2"], ln2, rstd, nmr, mv8, 0)
            d["ln2"] = ln2
            del d["pu2"]

        def ln_driver(i):
            if 1 <= i and i - 1 < NG:
                s2_lne(i - 1)
            if 2 <= i and i - 2 < NG:
                s4_lnu(i - 2)

        def s3_update(g):
            d = st_grp[g]
            mt = st_macro[g // M]
            esl = d["esl"]
            pu1 = psB.tile([128, EPG], F32, tag="pB")
            nc.tensor.matmul(
                pu1[:], lhsT=fWu1a[:], rhs=d["ln1T"][:], start=True, stop=False
            )
            nc.tensor.matmul(
                pu1[:], lhsT=sWu1b[:], rhs=mt["sf"][:, esl], start=False, stop=False
            )
            nc.tensor.matmul(
                pu1[:], lhsT=sWu1c[:], rhs=mt["rf"][:, esl], start=False, stop=True
            )
            del d["ln1T"]
            yu = work.tile([128, EPG], BF16, tag="yu")
            nc.scalar.activation(yu[:], pu1[:], AF.Silu, bias=fbu1[:], scale=1.0)
            pu2 = psA.tile([128, 4, 128], F32, tag="pA")
            emit_l2(pu2, yu, sWu2, None if zero_bias else sbu2_4)
            d["pu2"] = pu2

        def s5_agg(g):
            d = st_grp[g]
            mt = st_macro[g // M]
            gm = g % M
            Sps = psS.tile([128, 2 * G], F32, tag="pS")
            for c in range(4):
                nc.tensor.matmul(
                    Sps[:, 0:G],
                    lhsT=d["ln1"][:, c, :],
                    rhs=mt["oh"][:, (gm * 4 + c) * G : (gm * 4 + c + 1) * G],
                    start=(c == 0),
                    stop=(c == 3),
                )
            for c in range(4):
                nc.tensor.matmul(
                    Sps[:, G : 2 * G],
                    lhsT=d["ln2"][:, c, :],
                    rhs=mt["oh"][:, (gm * 4 + c) * G : (gm * 4 + c + 1) * G],
                    start=(c == 0),
                    stop=(c == 3),
                )
            nc.vector.tensor_copy(
                out=aggS1[:, g * G : (g + 1) * G], in_=Sps[:, 0:G]
            )
            nc.scalar.copy(
                out=aggS2[:, g * G : (g + 1) * G], in_=Sps[:, G : 2 * G]
            )
            del st_grp[g]

        NMAC = (NG + M - 1) // M
        macro_load(0)
        for i in range(NG + 3):
            ln_driver(i)
            if i >= 3:
                s5_agg(i - 3)
            if i < NG:
                if i % M == 0 and (i // M) + 1 < NMAC:
                    macro_load(i // M + 1)
                s1_embed(i)
            if i >= 1 and i - 1 < NG:
                s3_update(i - 1)

        # ---------- pnode phase (software-pipelined across blocks) ----------
        OBW = 4  # blocks per staging window
        st_blk = {}

        def win_load(wi):
            w = {}
            w["inv"] = macroP.tile([1, OBW * PB], F32, tag="invW", name="invW")
            nc.sync.dma_start(
                out=w["inv"][:], in_=inv_d[:, wi * OBW * PB : (wi + 1) * OBW * PB]
            )
            w["m01"] = macroP.tile([1, OBW * PB], BF16, tag="m01W", name="m01W")
            nc.sync.dma_start(
                out=w["m01"][:], in_=m01_d[:, wi * OBW * PB : (wi + 1) * OBW * PB]
            )
            st_blk["w%d" % wi] = w

        def p1_front(j):
            d = st_blk.setdefault(j, {})
            sl = slice(j * PB, (j + 1) * PB)
            d["sl"] = sl
            if j % OBW == 0:
                if j == 0:
                    win_load(0)
                if j // OBW + 1 < NPB // OBW:
                    win_load(j // OBW + 1)
            w = st_blk["w%d" % (j // OBW)]
            wsl = slice((j % OBW) * PB, (j % OBW + 1) * PB)

            pinv = psT.tile([128, PB], F32, tag="pT")
            nc.tensor.matmul(
                pinv[:], lhsT=ones_r32[:], rhs=w["inv"][:, wsl],
                start=True, stop=True,
            )
            invb = work.tile([128, PB], F32, tag="invb")
            nc.scalar.copy(out=invb[:], in_=pinv[:])

            pA = psB.tile([128, PB], F32, tag="pB")
            nc.tensor.matmul(
                pA[:], lhsT=fWp1ge[:], rhs=aggS1[:, sl], start=True, stop=False
            )
            nc.tensor.matmul(
                pA[:], lhsT=fWp1gu[:], rhs=aggS2[:, sl], start=False, stop=True
            )
            tA = work.tile([128, PB], BF16, tag="tA")
            nc.vector.tensor_tensor(out=tA[:], in0=pA[:], in1=invb[:], op=ALU.mult)

            pzp = psB.tile([128, PB], F32, tag="pB")
            nc.tensor.matmul(
                pzp[:], lhsT=sWp1n[:], rhs=pnT[:, sl], start=True, stop=False
            )
            nc.tensor.matmul(
                pzp[:], lhsT=ident16[:], rhs=tA[:], start=False, stop=False
            )
            nc.tensor.matmul(
                pzp[:], lhsT=bpe_row[:], rhs=w["m01"][:, wsl],
                start=False, stop=True,
            )
            yp = work.tile([128, PB], BF16, tag="yp")
            nc.scalar.activation(yp[:], pzp[:], AF.Silu, bias=sbp1[:], scale=1.0)

            pp2 = psA.tile([128, 4, 128], F32, tag="pA")
            emit_l2(pp2, yp, sWp2, None if zero_bias else sbp2_4)
            d["pp2"] = pp2

        def p2_ln(j):
            d = st_blk[j]
            mv8 = small.tile([128, 8, 2], F32, tag="mv8", name="mv8")
            ln_stats(d["pp2"], mv8, 0, "p")
            rstd, nmr = ln_rsqrt(mv8, 4, "p")
            lnp = work3.tile([128, 4, 128], BF16, tag="ln1", name="lnp")
            ln_apply(d["pp2"], lnp, rstd, nmr, mv8, 0)
            del d["pp2"]
            lnpT = work.tile([128, PB], BF16, tag="ln1T", name="lnpT")
            transp(lnp, lnpT)
            d["lnpT"] = lnpT

        def p3_out(j):
            d = st_blk[j]
            sl = d["sl"]
            lnpT = d["lnpT"]
            pzo = psB.tile([128, PB], F32, tag="pB")
            nc.tensor.matmul(
                pzo[:], lhsT=fWo1[:], rhs=lnpT[:], start=True, stop=False
            )
            nc.tensor.matmul(
                pzo[:], lhsT=sWo116[:], rhs=pnT[:, sl], start=False, stop=True
            )
            yo = work.tile([128, PB], BF16, tag="yo")
            nc.scalar.activation(yo[:], pzo[:], AF.Silu, bias=fbo1[:], scale=1.0)

            po = psS.tile([OUT, PB], F32, tag="pS")
            nc.tensor.matmul(
                po[:], lhsT=sWo2[:], rhs=yo[:], start=True, stop=zero_bias
            )
            if not zero_bias:
                nc.tensor.matmul(
                    po[:], lhsT=sbo2[:], rhs=ones_r[:], start=False, stop=True
                )
            if j % OBW == 0:
                d2 = st_blk.setdefault("ob%d" % (j // OBW), {})
                d2["ob"] = macroP.tile([OUT, OBW * PB], F32, tag="ob", name="ob")
            ob = st_blk["ob%d" % (j // OBW)]["ob"]
            nc.vector.tensor_copy(
                out=ob[:, (j % OBW) * PB : (j % OBW + 1) * PB], in_=po[:]
            )
            if j % OBW == OBW - 1:
                nc.sync.dma_start(
                    out=outT[:, (j - OBW + 1) * PB : (j + 1) * PB], in_=ob[:]
                )
            del st_blk[j]

        for j in range(NPB + 2):
            if j >= 1 and j - 1 < NPB:
                p2_ln(j - 1)
            if j >= 2:
                p3_out(j - 2)
            if j < NPB:
                p1_front(j)

    nc.compile()
    return nc


def _prep_core(ef_b, snd_b, rcv_b, rn_b, pn_b, tau_b, q):
    import ml_dtypes

    lo = q * QP
    mask = (rcv_b >= lo) & (rcv_b < lo + QP)
    ed = np.nonzero(mask)[0]
    loc = (rcv_b[ed] - lo).astype(np.int64)
    order = np.argsort(loc, kind="stable")
    ed, loc = ed[order], loc[order]
    grp = loc // G
    cnts = np.bincount(grp, minlength=NG)
    assert cnts.max() <= EPG, f"group overflow: {cnts.max()} > {EPG}"
    gstart = np.concatenate([[0], np.cumsum(cnts)[:-1]])
    slot = grp * EPG + (np.arange(len(ed)) - gstart[grp])

    efp = np.zeros((NEP, EIN), np.float32)
    efp[slot] = ef_b[ed]
    # host pre-gather of sender/receiver features per edge slot
    sfp = np.zeros((NEP, F), np.float32)
    sfp[slot] = rn_b[snd_b[ed]]
    rfp = np.zeros((NEP, F), np.float32)
    rfp[slot] = pn_b[rcv_b[ed]]
    rrel = np.full(NEP, -1.0, np.float32)
    rrel[slot] = (loc - grp * G).astype(np.float32)
    # precomputed one-hot [slot -> segment] per group, laid out
    # [128 partitions, NG, 4 chunks, G] with slot = chunk*128 + partition
    ohm = (
        rrel.reshape(NG, 4, 128, 1) == np.arange(G, dtype=np.float32)
    ).astype(np.float32)
    ohm = ohm.transpose(2, 0, 1, 3).reshape(128, NG * 4 * G)

    cnt = np.bincount(loc, minlength=QP).astype(np.float32)
    inv = (1.0 / np.maximum(cnt, 1.0)).astype(np.float32)
    m01 = np.minimum(cnt, 1.0)

    pn_q = pn_b[lo : lo + QP]
    return {
        "efT": np.ascontiguousarray(efp.T.astype(ml_dtypes.bfloat16)),
        "sfT": np.ascontiguousarray(sfp.T.astype(ml_dtypes.bfloat16)),
        "rfT": np.ascontiguousarray(rfp.T.astype(ml_dtypes.bfloat16)),
        "ohM": np.ascontiguousarray(ohm).astype(ml_dtypes.bfloat16),
        "pnT": np.ascontiguousarray(pn_q.T).astype(ml_dtypes.bfloat16),
        "invQ": inv.reshape(1, QP),
        "m01Q": m01.reshape(1, QP).astype(ml_dtypes.bfloat16),
        "tau": tau_b.reshape(1, 1).astype(np.float32),
    }


def _prep_weights(i):
    w = {
        "We1": i["We1"], "be1": i["be1"].reshape(F, 1), "We2": i["We2"],
        "be2_4": np.tile(i["be2"].reshape(1, F), (1, 4)),
        "Wu1a": i["Wu1"][0:F], "Wu1b": i["Wu1"][F : 2 * F],
        "Wu1c": i["Wu1"][2 * F : 3 * F],
        "bu1": i["bu1"].reshape(F, 1), "Wu2": i["Wu2"],
        "bu2_4": np.tile(i["bu2"].reshape(1, F), (1, 4)),
        "Wp1n": i["Wp1"][0:F], "Wp1g": i["Wp1"][F : 2 * F],
        "bp1": i["bp1"].reshape(F, 1), "Wp2": i["Wp2"],
        "bp2_4": np.tile(i["bp2"].reshape(1, F), (1, 4)),
        "Wo1": i["Wo1"], "bo1": i["bo1"].reshape(F, 1), "Wo2": i["Wo2"],
        "bo2": i["bo2"].reshape(1, OUT),
    }
    for k in ("e", "u", "p"):
        C1, c1 = i[f"C{k}1"], i[f"c{k}1"]
        C2, c2 = i[f"C{k}2"], i[f"c{k}2"]
        w[f"C{k}1"] = C1.reshape(1, H)
        w[f"c{k}1"] = c1.reshape(H, 1)
        w[f"C{k}2a"] = np.ascontiguousarray(C2[:, 0:F])
        w[f"C{k}2b"] = np.ascontiguousarray(C2[:, F : 2 * F])
        w[f"c{k}2a1"] = (c2[0:F] + 1.0).reshape(F, 1)
        w[f"c{k}2b"] = c2[F : 2 * F].reshape(F, 1)
    import ml_dtypes

    bf16_keys = {"We1", "We2", "Wu1b", "Wu1c", "Wu2", "Wp1n", "Wp2", "Wo2",
                 "be2_4", "bu2_4", "bp2_4", "bo2"}
    return {
        k: np.ascontiguousarray(
            v, dtype=ml_dtypes.bfloat16 if k in bf16_keys else np.float32
        )
        for k, v in w.items()
    }


_NC_CACHE = {}


def _zero_bias(i):
    return all(
        float(np.abs(np.asarray(i[k])).max()) == 0.0
        for k in ("be2", "bu2", "bp2", "bo2")
    )


def build_in_maps(inputs):
    i = {k: np.asarray(v) for k, v in inputs.items()}
    w = _prep_weights(i)
    in_maps = []
    for core in range(8):
        b, q = core // NQ, core % NQ
        m = dict(w)
        m.update(
            _prep_core(
                i["edge_features"][b], i["senders"][b], i["receivers"][b],
                i["rnode_features"][b], i["pnode_features"][b], i["tau"][b], q
            )
        )
        in_maps.append(m)
    return in_maps


def get_nc(zero_bias=True):
    key = ("nc", bool(zero_bias))
    if key not in _NC_CACHE:
        _NC_CACHE[key] = _build_nc(bool(zero_bias))
    return _NC_CACHE[key]


def assemble(results):
    out = np.zeros((B, NPTOT, OUT), np.float32)
    for core in range(8):
        b, q = core // NQ, core % NQ
        out[b, q * QP : (q + 1) * QP, :] = results[core]["outT"].T
    return out


def kernel(**inputs):
    from concourse.bass_utils import run_bass_kernel_spmd

    i = {k: np.asarray(v) for k, v in inputs.items()}
    nc = get_nc(_zero_bias(i))
    in_maps = build_in_maps(i)
    res = run_bass_kernel_spmd(nc, in_maps, list(range(8)))
    return assemble(res.results)


if __name__ == "__main__":
    import reference

    inputs = reference.setup_inputs()
    out = kernel(**{k: np.asarray(v) for k, v in inputs.items()})
    print("out", out.shape, out.dtype)
